# revision 8
# baseline (speedup 1.0000x reference)
"""Trainium2 Bass kernel for the CerealBar VIN problem.

Self-contained: hardcodes shapes B=512, E=25, 6 orientations, gamma=0.9,
8-core batch sharding (64 samples/core).

Math (derived from the reference, validated bit-exact in numpy):
  The grouped 3x3 conv is a set of one-hot spatial shifts. With
  m = (1-obstacles_axial) * axial_mask (0/1), gm[o] = goals_axial[o]*m:
    RT0[o] = m * sh_{d0(o)}(gm[o]);  RT1[o] = m * sh_{d1(o)}(gm[o])
    v0[o]  = max(RT0[o], RT1[o], gm[(o+1)%6], gm[(o+5)%6])
    repeat n-1 times:
      T0[o] = m * g * sh_{d0(o)}(v[o]);  T1[o] = m * g * sh_{d1(o)}(v[o])
      W[o]  = g*v[o] + gm[o]
      v[o]  = max(T0[o]+RT0[o], T1[o]+RT1[o], W[(o+1)%6], W[(o+5)%6])
    Q0 = T0+RT0, Q1 = T1+RT1, W = g*v+gm   (from final v)
    out[b] = [Q0[rot,uu,vv], Q1[rot,uu,vv], W[(rot+1)%6][uu,vv],
              W[(rot+5)%6][uu,vv]],  rot=(alpha+1)%6, uu=u-v//2+12

Device layout: partition p = h*64 + i -> sample i of the core, row-half h.
Each (orientation, half) plane = 21 rows x 26 cols flat (546): row 0 top
halo, rows 1..19 data, row 20 bottom halo, col 25 zero pad. half0 data =
grid rows 0..18, half1 = grid rows 19..37 (37 is a fake all-masked row).
Shifts are free-dim AP offsets (d = 26*dy+dx); the row-split halo rows are
refreshed once per iteration by two cross-partition SBUF-SBUF DMAs.
"""
import json
import sys

sys.path.insert(0, "/opt/trn_rl_repo")

import numpy as np

import concourse.bass as bass
import concourse.mybir as mybir
from concourse.ap import AP
from concourse.bass_utils import run_bass_kernel_spmd
from concourse.tile import TileContext

E = 25
ADD = 12
GAMMA = 0.9
PR = 40          # padded full-grid rows (grid rows -1..38 at idx r+1)
PC = 26
SLOT = 546       # 21 * 26 per half-plane
DOF = 26         # data offset within a slot (row 1)
DN = 494         # data elems (rows 1..19)
N_CORES = 8
BPC = 64         # samples per core

# shifts (dy, dx): out[y, x] = in[y+dy, x+dx]
D0 = [(0, 1), (1, 0), (1, -1), (0, -1), (-1, 0), (-1, 1)]
D1 = [(0, -1), (-1, 0), (-1, 1), (0, 1), (1, 0), (1, -1)]
PAIRS = [(1, 2), (4, 5), (0, 3)]

import os as _os

if _os.environ.get("KDT", "fp16") == "fp16":
    DTYPE = mybir.dt.float16
    NP_DT = np.float16
else:
    DTYPE = mybir.dt.float32
    NP_DT = np.float32

TRACE = False
LAST_RESULT = None

_u = np.arange(E)[:, None]
_v = np.arange(E)[None, :]
_ROW = (_u - _v // 2 + ADD) + 1
_COL = np.broadcast_to(_v, (E, E))


# ---------------------------------------------------------------- BIR fixups
def _split_multi_waits(bir):
    """The installed walrus rejects >1 sync wait per instruction; hoist
    extras onto single-wait NoOps inserted before it on the same engine."""
    for fn in bir.get("functions", []):
        for blk in fn.get("blocks", []):
            out = []
            for ins in blk.get("instructions", []):
                si = ins.get("sync_info")
                waits = (si or {}).get("on_wait") or []
                if len(waits) > 1:
                    for k, w in enumerate(waits[:-1]):
                        out.append({
                            "debug": ins.get("debug", 0),
                            "engine": ins["engine"],
                            "ins": [], "outs": [],
                            "name": f"{ins['name']}_w{k}",
                            "opcode": "NoOp",
                            "sync_info": {"on_wait": [w], "on_update": []},
                            "text_hint": "split_wait",
                        })
                    si["on_wait"] = [waits[-1]]
                out.append(ins)
            blk["instructions"] = out
    return bir


def _install_compat(nc):
    orig = nc.to_json_bytes

    def patched():
        return json.dumps(_split_multi_waits(json.loads(orig()))).encode()

    nc.to_json_bytes = patched


# ---------------------------------------------------------------- kernel build
def _rap(t, off, pairs):
    """Raw AP over pool tile t (full 128 partitions) with free dims pairs."""
    return AP(t.tensor, int(t.offset) + off, [list(t.ap[0])] + [list(p) for p in pairs])


def _delta(d):
    return 26 * d[0] + d[1]


def _emit_shift_group(nc, dst, src, src_slot0, dlist, m_sb, scalar, rt=None):
    """dst[o] = (sh_{dlist[o]}(src[slot o+src_slot0]) * scalar) * m, for the
    three plane-pairs. If scalar is None, plain mult by m (RT build)."""
    mul = mybir.AluOpType.mult
    for oa, ob in PAIRS:
        da, db = _delta(dlist[oa]), _delta(dlist[ob])
        step = (ob - oa) * SLOT + (db - da)
        in0 = _rap(src, (oa + src_slot0) * SLOT + DOF + da, [[step, 2], [1, DN]])
        out = _rap(dst, oa * SLOT + DOF, [[(ob - oa) * SLOT, 2], [1, DN]])
        m_b = _rap(m_sb, DOF, [[0, 2], [1, DN]])
        if scalar is None:
            nc.vector.tensor_tensor(out=out, in0=in0, in1=m_b, op=mul)
        else:
            nc.vector.scalar_tensor_tensor(
                out=out, in0=in0, scalar=scalar, in1=m_b, op0=mul, op1=mul)


def build_nc(n_iter):
    nc = bass.Bass()
    _install_compat(nc)
    mx = mybir.AluOpType.max
    mul = mybir.AluOpType.mult
    add = mybir.AluOpType.add

    goals_d = nc.declare_dram_parameter("goals", [128, 6, SLOT], DTYPE, isOutput=False)
    m_d = nc.declare_dram_parameter("m", [128, SLOT], DTYPE, isOutput=False)
    q0_d = nc.declare_dram_parameter("q0", [128, 6, SLOT], DTYPE, isOutput=True)
    q1_d = nc.declare_dram_parameter("q1", [128, 6, SLOT], DTYPE, isOutput=True)
    w_d = nc.declare_dram_parameter("w", [128, 6, SLOT], DTYPE, isOutput=True)

    with TileContext(nc) as tc:
        with tc.tile_pool(name="p", bufs=1) as pool:
            m_sb = pool.tile([128, SLOT], DTYPE)
            gme = pool.tile([128, 6, SLOT], DTYPE)
            v = pool.tile([128, 6, SLOT], DTYPE)
            gv = pool.tile([128, 6, SLOT], DTYPE)     # gamma * v (ACT)
            t0 = pool.tile([128, 6, SLOT], DTYPE)
            t1 = pool.tile([128, 6, SLOT], DTYPE)
            we = pool.tile([128, 8, SLOT], DTYPE)     # W slots 1..6, dups 0/7

            nc.gpsimd.memset(v[:], 0.0)
            nc.gpsimd.memset(gv[:], 0.0)
            nc.gpsimd.memset(we[:], 0.0)
            nc.sync.dma_start(out=m_sb[:], in_=m_d[:])
            goals_sb = pool.tile([128, 6, SLOT], DTYPE)
            nc.sync.dma_start(out=goals_sb[:], in_=goals_d[:])

            # gm = goals * m (full slots incl halo rows; host fills halos)
            m_b6 = _rap(m_sb, 0, [[0, 6], [1, SLOT]])
            nc.vector.tensor_tensor(out=gme[:], in0=goals_sb[:], in1=m_b6, op=mul)

            dv = (slice(None), slice(None), slice(DOF, DOF + DN))  # data view

            def halo_we():
                # half1 top halo <- half0 grid row 18 (buffer row 19)
                nc.sync.dma_start(out=we[64:128, 1:7, 0:26],
                                  in_=we[0:64, 1:7, 19 * 26:20 * 26])
                # half0 bottom halo <- half1 grid row 19 (buffer row 1)
                nc.sync.dma_start(out=we[0:64, 1:7, 20 * 26:21 * 26],
                                  in_=we[64:128, 1:7, 26:52])

            BROW = 19 * 26  # buffer row 19 offset

            def w_build(first):
                # gv = gamma*v on ACT (parallel engine; STT on DVE has no
                # fp16 fast mode). Then W = gv + gm on DVE, boundary rows
                # (1, 19) first so the halo DMAs overlap the interior add.
                if not first:
                    nc.scalar.activation(
                        out=gv[:], in_=v[:],
                        func=mybir.ActivationFunctionType.Copy, scale=GAMMA)
                # rows 1 and 19 of every slot: [[546,6],[468,2],[1,26]]
                wa = _rap(we, SLOT + 26, [[SLOT, 6], [468, 2], [1, 26]])
                ga = _rap(gv, 26, [[SLOT, 6], [468, 2], [1, 26]])
                ma = _rap(gme, 26, [[SLOT, 6], [468, 2], [1, 26]])
                nc.vector.tensor_tensor(out=wa, in0=ga, in1=ma, op=add)
                halo_we()
                # interior rows 2..18 (contiguous 442) of every slot
                wb = _rap(we, SLOT + 52, [[SLOT, 6], [1, 442]])
                gb = _rap(gv, 52, [[SLOT, 6], [1, 442]])
                mb = _rap(gme, 52, [[SLOT, 6], [1, 442]])
                nc.vector.tensor_tensor(out=wb, in0=gb, in1=mb, op=add)

            def dups():
                nc.scalar.copy(out=we[:, 0, DOF:DOF + DN],
                               in_=we[:, 6, DOF:DOF + DN])
                nc.scalar.copy(out=we[:, 7, DOF:DOF + DN],
                               in_=we[:, 1, DOF:DOF + DN])

            def t_build():
                _emit_shift_group(nc, t0, we, 1, D0, m_sb, None)
                _emit_shift_group(nc, t1, we, 1, D1, m_sb, None)

            def x_build():
                # X[o] = max(sh_{+d0(o)}(W[o]), sh_{-d0(o)}(W[o])) -> t0
                # (D1 = -D0, so max(T0,T1) = m * X)
                for oa, ob in PAIRS:
                    da, db = _delta(D0[oa]), _delta(D0[ob])
                    in0 = _rap(we, (oa + 1) * SLOT + DOF + da,
                               [[(ob - oa) * SLOT + (db - da), 2], [1, DN]])
                    in1 = _rap(we, (oa + 1) * SLOT + DOF - da,
                               [[(ob - oa) * SLOT - (db - da), 2], [1, DN]])
                    out = _rap(t0, oa * SLOT + DOF, [[(ob - oa) * SLOT, 2], [1, DN]])
                    nc.vector.tensor_tensor(out=out, in0=in0, in1=in1, op=mx)

            m_b6d = _rap(m_sb, DOF, [[0, 6], [1, DN]])

            for it in range(n_iter):
                w_build(it == 0)
                dups()
                x_build()
                nc.vector.tensor_tensor(out=t0[dv], in0=t0[dv], in1=m_b6d, op=mul)
                nc.vector.tensor_tensor(
                    out=t1[dv], in0=we[:, 2:8, DOF:DOF + DN],
                    in1=we[:, 0:6, DOF:DOF + DN], op=mx)
                nc.vector.tensor_tensor(out=v[dv], in0=t0[dv], in1=t1[dv], op=mx)

            # final partial: Q0 = m*sh0(W), Q1 = m*sh1(W), q2/q3 = W views
            w_build(False)
            t_build()

            nc.sync.dma_start(out=q0_d[:], in_=t0[:])
            nc.sync.dma_start(out=q1_d[:], in_=t1[:])
            nc.sync.dma_start(out=w_d[:], in_=we[:, 1:7])
    return nc


_NC_CACHE = {}


def _get_nc(n_iter):
    if n_iter not in _NC_CACHE:
        _NC_CACHE[n_iter] = build_nc(n_iter)
    return _NC_CACHE[n_iter]


# ---------------------------------------------------------------- host side
def _to_padded_axial(x):
    out = np.zeros(x.shape[:-2] + (PR, PC), np.float32)
    out[..., _ROW, _COL] = x
    return out


def kernel(offset_input_goals, offset_current_state, offset_obstacles,
           num_iterations):
    global LAST_RESULT
    goals = np.asarray(offset_input_goals, np.float32)
    state = np.asarray(offset_current_state)
    obst = np.asarray(offset_obstacles, np.float32)
    n_iter = int(num_iterations)
    B = goals.shape[0]
    assert B == N_CORES * BPC and n_iter >= 1

    goals_ax = _to_padded_axial(goals)                     # [B,6,40,26]
    mask = _to_padded_axial(np.ones((E, E), np.float32))
    m_full = (1.0 - _to_padded_axial(obst)) * mask         # [B,40,26]

    def split(x):  # [B, ..., 40, 26] -> [B, 2, ..., 546]
        h0 = x[..., 0:21, :].reshape(x.shape[:-2] + (SLOT,))
        h1 = x[..., 19:40, :].reshape(x.shape[:-2] + (SLOT,))
        return h0, h1

    g0, g1 = split(goals_ax)
    m0, m1 = split(m_full)

    in_maps = []
    for c in range(N_CORES):
        s = slice(c * BPC, (c + 1) * BPC)
        in_maps.append({
            "goals": np.concatenate([g0[s], g1[s]], 0).astype(NP_DT),
            "m": np.concatenate([m0[s], m1[s]], 0).astype(NP_DT),
        })

    nc = _get_nc(n_iter)
    res = run_bass_kernel_spmd(nc, in_maps, core_ids=list(range(N_CORES)),
                               trace=TRACE)
    LAST_RESULT = res

    out = np.zeros((B, 4), np.float32)
    alpha = state[:, 0].astype(np.int64)
    uu = (state[:, 1] - state[:, 2] // 2 + ADD).astype(np.int64)
    vv = state[:, 2].astype(np.int64)
    rot = (alpha + 1) % 6
    h = (uu > 18).astype(np.int64)
    r = np.where(h == 0, uu + 1, uu - 18)
    idx = r * 26 + vv
    for c in range(N_CORES):
        rr = res.results[c]
        q0 = np.asarray(rr["q0"], np.float32)
        q1 = np.asarray(rr["q1"], np.float32)
        w = np.asarray(rr["w"], np.float32)
        bs = np.arange(c * BPC, (c + 1) * BPC)
        p = h[bs] * 64 + np.arange(BPC)
        out[bs, 0] = q0[p, rot[bs], idx[bs]]
        out[bs, 1] = q1[p, rot[bs], idx[bs]]
        out[bs, 2] = w[p, (rot[bs] + 1) % 6, idx[bs]]
        out[bs, 3] = w[p, (rot[bs] + 5) % 6, idx[bs]]
    return out


# revision 9
# speedup vs baseline: 1.1619x; 1.1619x over previous
"""Trainium2 Bass kernel for the CerealBar VIN problem.

Self-contained: hardcodes shapes B=512, E=25, 6 orientations, gamma=0.9,
8-core batch sharding (64 samples/core).

Math (derived from the reference, validated bit-exact in numpy):
  The grouped 3x3 conv is a set of one-hot spatial shifts. With
  m = (1-obstacles_axial) * axial_mask (0/1), gm[o] = goals_axial[o]*m:
    RT0[o] = m * sh_{d0(o)}(gm[o]);  RT1[o] = m * sh_{d1(o)}(gm[o])
    v0[o]  = max(RT0[o], RT1[o], gm[(o+1)%6], gm[(o+5)%6])
    repeat n-1 times:
      T0[o] = m * g * sh_{d0(o)}(v[o]);  T1[o] = m * g * sh_{d1(o)}(v[o])
      W[o]  = g*v[o] + gm[o]
      v[o]  = max(T0[o]+RT0[o], T1[o]+RT1[o], W[(o+1)%6], W[(o+5)%6])
    Q0 = T0+RT0, Q1 = T1+RT1, W = g*v+gm   (from final v)
    out[b] = [Q0[rot,uu,vv], Q1[rot,uu,vv], W[(rot+1)%6][uu,vv],
              W[(rot+5)%6][uu,vv]],  rot=(alpha+1)%6, uu=u-v//2+12

Device layout: partition p = h*64 + i -> sample i of the core, row-half h.
Each (orientation, half) plane = 21 rows x 26 cols flat (546): row 0 top
halo, rows 1..19 data, row 20 bottom halo, col 25 zero pad. half0 data =
grid rows 0..18, half1 = grid rows 19..37 (37 is a fake all-masked row).
Shifts are free-dim AP offsets (d = 26*dy+dx); the row-split halo rows are
refreshed once per iteration by two cross-partition SBUF-SBUF DMAs.
"""
import json
import sys

sys.path.insert(0, "/opt/trn_rl_repo")

import numpy as np

import concourse.bass as bass
import concourse.mybir as mybir
from concourse.ap import AP
from concourse.bass_utils import run_bass_kernel_spmd
from concourse.tile import TileContext

E = 25
ADD = 12
GAMMA = 0.9
PR = 40          # padded full-grid rows (grid rows -1..38 at idx r+1)
PC = 26
SLOT = 546       # 21 * 26 per half-plane
DOF = 26         # data offset within a slot (row 1)
DN = 494         # data elems (rows 1..19)
N_CORES = 8
BPC = 64         # samples per core

# shifts (dy, dx): out[y, x] = in[y+dy, x+dx]
D0 = [(0, 1), (1, 0), (1, -1), (0, -1), (-1, 0), (-1, 1)]
D1 = [(0, -1), (-1, 0), (-1, 1), (0, 1), (1, 0), (1, -1)]
PAIRS = [(1, 2), (4, 5), (0, 3)]

import os as _os

if _os.environ.get("KDT", "fp16") == "fp16":
    DTYPE = mybir.dt.float16
    NP_DT = np.float16
else:
    DTYPE = mybir.dt.float32
    NP_DT = np.float32

TRACE = False
LAST_RESULT = None

_u = np.arange(E)[:, None]
_v = np.arange(E)[None, :]
_ROW = (_u - _v // 2 + ADD) + 1
_COL = np.broadcast_to(_v, (E, E))


# ---------------------------------------------------------------- BIR fixups
def _split_multi_waits(bir):
    """The installed walrus rejects >1 sync wait per instruction; hoist
    extras onto single-wait NoOps inserted before it on the same engine."""
    for fn in bir.get("functions", []):
        for blk in fn.get("blocks", []):
            out = []
            for ins in blk.get("instructions", []):
                si = ins.get("sync_info")
                waits = (si or {}).get("on_wait") or []
                if len(waits) > 1:
                    for k, w in enumerate(waits[:-1]):
                        out.append({
                            "debug": ins.get("debug", 0),
                            "engine": ins["engine"],
                            "ins": [], "outs": [],
                            "name": f"{ins['name']}_w{k}",
                            "opcode": "NoOp",
                            "sync_info": {"on_wait": [w], "on_update": []},
                            "text_hint": "split_wait",
                        })
                    si["on_wait"] = [waits[-1]]
                out.append(ins)
            blk["instructions"] = out
    return bir


def _install_compat(nc):
    orig = nc.to_json_bytes

    def patched():
        return json.dumps(_split_multi_waits(json.loads(orig()))).encode()

    nc.to_json_bytes = patched


# ---------------------------------------------------------------- kernel build
def _rap(t, off, pairs):
    """Raw AP over pool tile t (full 128 partitions) with free dims pairs."""
    return AP(t.tensor, int(t.offset) + off, [list(t.ap[0])] + [list(p) for p in pairs])


def _delta(d):
    return 26 * d[0] + d[1]


def _emit_shift_group(nc, dst, src, src_slot0, dlist, m_sb, scalar, rt=None):
    """dst[o] = (sh_{dlist[o]}(src[slot o+src_slot0]) * scalar) * m, for the
    three plane-pairs. If scalar is None, plain mult by m (RT build)."""
    mul = mybir.AluOpType.mult
    for oa, ob in PAIRS:
        da, db = _delta(dlist[oa]), _delta(dlist[ob])
        step = (ob - oa) * SLOT + (db - da)
        in0 = _rap(src, (oa + src_slot0) * SLOT + DOF + da, [[step, 2], [1, DN]])
        out = _rap(dst, oa * SLOT + DOF, [[(ob - oa) * SLOT, 2], [1, DN]])
        m_b = _rap(m_sb, DOF, [[0, 2], [1, DN]])
        if scalar is None:
            nc.vector.tensor_tensor(out=out, in0=in0, in1=m_b, op=mul)
        else:
            nc.vector.scalar_tensor_tensor(
                out=out, in0=in0, scalar=scalar, in1=m_b, op0=mul, op1=mul)


def build_nc(n_iter):
    nc = bass.Bass()
    _install_compat(nc)
    mx = mybir.AluOpType.max
    mul = mybir.AluOpType.mult
    add = mybir.AluOpType.add

    goals_d = nc.declare_dram_parameter("goals", [128, 6, SLOT], DTYPE, isOutput=False)
    m_d = nc.declare_dram_parameter("m", [128, SLOT], DTYPE, isOutput=False)
    q0_d = nc.declare_dram_parameter("q0", [128, 6, SLOT], DTYPE, isOutput=True)
    q1_d = nc.declare_dram_parameter("q1", [128, 6, SLOT], DTYPE, isOutput=True)
    w_d = nc.declare_dram_parameter("w", [128, 6, SLOT], DTYPE, isOutput=True)

    with TileContext(nc) as tc:
        with tc.tile_pool(name="p", bufs=1) as pool:
            m_sb = pool.tile([128, SLOT], DTYPE)
            gme = pool.tile([128, 6, SLOT], DTYPE)
            v = pool.tile([128, 6, SLOT], DTYPE)
            gv = pool.tile([128, 6, SLOT], DTYPE)     # gamma * v (ACT)
            t0 = pool.tile([128, 6, SLOT], DTYPE)
            t1 = pool.tile([128, 6, SLOT], DTYPE)
            we = pool.tile([128, 8, SLOT], DTYPE)     # W slots 1..6, dups 0/7

            nc.gpsimd.memset(v[:], 0.0)
            nc.gpsimd.memset(gv[:], 0.0)
            nc.gpsimd.memset(we[:], 0.0)
            nc.sync.dma_start(out=m_sb[:], in_=m_d[:])
            goals_sb = pool.tile([128, 6, SLOT], DTYPE)
            nc.sync.dma_start(out=goals_sb[:], in_=goals_d[:])

            # gm = goals * m (full slots incl halo rows; host fills halos)
            m_b6 = _rap(m_sb, 0, [[0, 6], [1, SLOT]])
            nc.vector.tensor_tensor(out=gme[:], in0=goals_sb[:], in1=m_b6, op=mul)

            dv = (slice(None), slice(None), slice(DOF, DOF + DN))  # data view

            def halo_we():
                # half1 top halo <- half0 grid row 18 (buffer row 19)
                nc.sync.dma_start(out=we[64:128, 1:7, 0:26],
                                  in_=we[0:64, 1:7, 19 * 26:20 * 26])
                # half0 bottom halo <- half1 grid row 19 (buffer row 1)
                nc.sync.dma_start(out=we[0:64, 1:7, 20 * 26:21 * 26],
                                  in_=we[64:128, 1:7, 26:52])

            BROW = 19 * 26  # buffer row 19 offset

            def w_build(first):
                # gv = gamma*v on ACT (parallel engine; STT on DVE has no
                # fp16 fast mode). Then W = gv + gm on DVE, boundary rows
                # (1, 19) first so the halo DMAs overlap the interior add.
                if not first:
                    # DVE tensor_scalar gets the 4x fp16 mode (single-src)
                    nc.vector.tensor_scalar_mul(out=gv[:], in0=v[:], scalar1=GAMMA)
                # rows 1 and 19 of every slot: [[546,6],[468,2],[1,26]]
                wa = _rap(we, SLOT + 26, [[SLOT, 6], [468, 2], [1, 26]])
                ga = _rap(gv, 26, [[SLOT, 6], [468, 2], [1, 26]])
                ma = _rap(gme, 26, [[SLOT, 6], [468, 2], [1, 26]])
                nc.vector.tensor_tensor(out=wa, in0=ga, in1=ma, op=add)
                halo_we()
                # interior rows 2..18 (contiguous 442) of every slot
                wb = _rap(we, SLOT + 52, [[SLOT, 6], [1, 442]])
                gb = _rap(gv, 52, [[SLOT, 6], [1, 442]])
                mb = _rap(gme, 52, [[SLOT, 6], [1, 442]])
                nc.vector.tensor_tensor(out=wb, in0=gb, in1=mb, op=add)

            def dups():
                nc.scalar.copy(out=we[:, 0, DOF:DOF + DN],
                               in_=we[:, 6, DOF:DOF + DN])
                nc.scalar.copy(out=we[:, 7, DOF:DOF + DN],
                               in_=we[:, 1, DOF:DOF + DN])

            def t_build():
                _emit_shift_group(nc, t0, we, 1, D0, m_sb, None)
                _emit_shift_group(nc, t1, we, 1, D1, m_sb, None)

            def x_build():
                # X[o] = max(sh_{+d0(o)}(W[o]), sh_{-d0(o)}(W[o])) -> t0
                # (D1 = -D0, so max(T0,T1) = m * X)
                for oa, ob in PAIRS:
                    da, db = _delta(D0[oa]), _delta(D0[ob])
                    in0 = _rap(we, (oa + 1) * SLOT + DOF + da,
                               [[(ob - oa) * SLOT + (db - da), 2], [1, DN]])
                    in1 = _rap(we, (oa + 1) * SLOT + DOF - da,
                               [[(ob - oa) * SLOT - (db - da), 2], [1, DN]])
                    out = _rap(t0, oa * SLOT + DOF, [[(ob - oa) * SLOT, 2], [1, DN]])
                    nc.vector.tensor_tensor(out=out, in0=in0, in1=in1, op=mx)

            m_b6d = _rap(m_sb, DOF, [[0, 6], [1, DN]])

            for it in range(n_iter):
                w_build(it == 0)
                dups()
                x_build()
                nc.vector.tensor_tensor(out=t0[dv], in0=t0[dv], in1=m_b6d, op=mul)
                nc.vector.tensor_tensor(
                    out=t1[dv], in0=we[:, 2:8, DOF:DOF + DN],
                    in1=we[:, 0:6, DOF:DOF + DN], op=mx)
                nc.vector.tensor_tensor(out=v[dv], in0=t0[dv], in1=t1[dv], op=mx)

            # final partial: Q0 = m*sh0(W), Q1 = m*sh1(W), q2/q3 = W views
            w_build(False)
            t_build()

            nc.sync.dma_start(out=q0_d[:], in_=t0[:])
            nc.sync.dma_start(out=q1_d[:], in_=t1[:])
            nc.sync.dma_start(out=w_d[:], in_=we[:, 1:7])
    return nc


_NC_CACHE = {}


def _get_nc(n_iter):
    if n_iter not in _NC_CACHE:
        _NC_CACHE[n_iter] = build_nc(n_iter)
    return _NC_CACHE[n_iter]


# ---------------------------------------------------------------- host side
def _to_padded_axial(x):
    out = np.zeros(x.shape[:-2] + (PR, PC), np.float32)
    out[..., _ROW, _COL] = x
    return out


def kernel(offset_input_goals, offset_current_state, offset_obstacles,
           num_iterations):
    global LAST_RESULT
    goals = np.asarray(offset_input_goals, np.float32)
    state = np.asarray(offset_current_state)
    obst = np.asarray(offset_obstacles, np.float32)
    n_iter = int(num_iterations)
    B = goals.shape[0]
    assert B == N_CORES * BPC and n_iter >= 1

    goals_ax = _to_padded_axial(goals)                     # [B,6,40,26]
    mask = _to_padded_axial(np.ones((E, E), np.float32))
    m_full = (1.0 - _to_padded_axial(obst)) * mask         # [B,40,26]

    def split(x):  # [B, ..., 40, 26] -> [B, 2, ..., 546]
        h0 = x[..., 0:21, :].reshape(x.shape[:-2] + (SLOT,))
        h1 = x[..., 19:40, :].reshape(x.shape[:-2] + (SLOT,))
        return h0, h1

    g0, g1 = split(goals_ax)
    m0, m1 = split(m_full)

    in_maps = []
    for c in range(N_CORES):
        s = slice(c * BPC, (c + 1) * BPC)
        in_maps.append({
            "goals": np.concatenate([g0[s], g1[s]], 0).astype(NP_DT),
            "m": np.concatenate([m0[s], m1[s]], 0).astype(NP_DT),
        })

    nc = _get_nc(n_iter)
    res = run_bass_kernel_spmd(nc, in_maps, core_ids=list(range(N_CORES)),
                               trace=TRACE)
    LAST_RESULT = res

    out = np.zeros((B, 4), np.float32)
    alpha = state[:, 0].astype(np.int64)
    uu = (state[:, 1] - state[:, 2] // 2 + ADD).astype(np.int64)
    vv = state[:, 2].astype(np.int64)
    rot = (alpha + 1) % 6
    h = (uu > 18).astype(np.int64)
    r = np.where(h == 0, uu + 1, uu - 18)
    idx = r * 26 + vv
    for c in range(N_CORES):
        rr = res.results[c]
        q0 = np.asarray(rr["q0"], np.float32)
        q1 = np.asarray(rr["q1"], np.float32)
        w = np.asarray(rr["w"], np.float32)
        bs = np.arange(c * BPC, (c + 1) * BPC)
        p = h[bs] * 64 + np.arange(BPC)
        out[bs, 0] = q0[p, rot[bs], idx[bs]]
        out[bs, 1] = q1[p, rot[bs], idx[bs]]
        out[bs, 2] = w[p, (rot[bs] + 1) % 6, idx[bs]]
        out[bs, 3] = w[p, (rot[bs] + 5) % 6, idx[bs]]
    return out


# revision 13
# speedup vs baseline: 1.2583x; 1.0830x over previous
"""Trainium2 Bass kernel for the CerealBar VIN problem.

Self-contained: hardcodes shapes B=512, E=25, 6 orientations, gamma=0.9,
8-core batch sharding (64 samples/core).

Math (derived from the reference, validated bit-exact in numpy):
  The grouped 3x3 conv is a set of one-hot spatial shifts. With
  m = (1-obstacles_axial) * axial_mask (0/1), gm[o] = goals_axial[o]*m:
    RT0[o] = m * sh_{d0(o)}(gm[o]);  RT1[o] = m * sh_{d1(o)}(gm[o])
    v0[o]  = max(RT0[o], RT1[o], gm[(o+1)%6], gm[(o+5)%6])
    repeat n-1 times:
      T0[o] = m * g * sh_{d0(o)}(v[o]);  T1[o] = m * g * sh_{d1(o)}(v[o])
      W[o]  = g*v[o] + gm[o]
      v[o]  = max(T0[o]+RT0[o], T1[o]+RT1[o], W[(o+1)%6], W[(o+5)%6])
    Q0 = T0+RT0, Q1 = T1+RT1, W = g*v+gm   (from final v)
    out[b] = [Q0[rot,uu,vv], Q1[rot,uu,vv], W[(rot+1)%6][uu,vv],
              W[(rot+5)%6][uu,vv]],  rot=(alpha+1)%6, uu=u-v//2+12

Device layout: partition p = h*64 + i -> sample i of the core, row-half h.
Each (orientation, half) plane = 21 rows x 26 cols flat (546): row 0 top
halo, rows 1..19 data, row 20 bottom halo, col 25 zero pad. half0 data =
grid rows 0..18, half1 = grid rows 19..37 (37 is a fake all-masked row).
Shifts are free-dim AP offsets (d = 26*dy+dx); the row-split halo rows are
refreshed once per iteration by two cross-partition SBUF-SBUF DMAs.
"""
import json
import sys

sys.path.insert(0, "/opt/trn_rl_repo")

import numpy as np

import concourse.bass as bass
import concourse.mybir as mybir
from concourse.ap import AP
from concourse.bass_utils import run_bass_kernel_spmd
from concourse.tile import TileContext

E = 25
ADD = 12
GAMMA = 0.9
PR = 40          # padded full-grid rows (grid rows -1..38 at idx r+1)
PC = 26
SLOT = 546       # 21 * 26 per half-plane
DOF = 26         # data offset within a slot (row 1)
DN = 494         # data elems (rows 1..19)
N_CORES = 8
BPC = 64         # samples per core

# shifts (dy, dx): out[y, x] = in[y+dy, x+dx]
D0 = [(0, 1), (1, 0), (1, -1), (0, -1), (-1, 0), (-1, 1)]
D1 = [(0, -1), (-1, 0), (-1, 1), (0, 1), (1, 0), (1, -1)]
PAIRS = [(1, 2), (4, 5), (0, 3)]

import os as _os

if _os.environ.get("KDT", "fp16") == "fp16":
    DTYPE = mybir.dt.float16
    NP_DT = np.float16
else:
    DTYPE = mybir.dt.float32
    NP_DT = np.float32

TRACE = False
LAST_RESULT = None

_u = np.arange(E)[:, None]
_v = np.arange(E)[None, :]
_ROW = (_u - _v // 2 + ADD) + 1
_COL = np.broadcast_to(_v, (E, E))


# ---------------------------------------------------------------- BIR fixups
def _split_multi_waits(bir):
    """The installed walrus rejects >1 sync wait per instruction; hoist
    extras onto single-wait NoOps inserted before it on the same engine."""
    for fn in bir.get("functions", []):
        for blk in fn.get("blocks", []):
            out = []
            for ins in blk.get("instructions", []):
                si = ins.get("sync_info")
                waits = (si or {}).get("on_wait") or []
                if len(waits) > 1:
                    for k, w in enumerate(waits[:-1]):
                        out.append({
                            "debug": ins.get("debug", 0),
                            "engine": ins["engine"],
                            "ins": [], "outs": [],
                            "name": f"{ins['name']}_w{k}",
                            "opcode": "NoOp",
                            "sync_info": {"on_wait": [w], "on_update": []},
                            "text_hint": "split_wait",
                        })
                    si["on_wait"] = [waits[-1]]
                out.append(ins)
            blk["instructions"] = out
    return bir


def _install_compat(nc):
    orig = nc.to_json_bytes

    def patched():
        return json.dumps(_split_multi_waits(json.loads(orig()))).encode()

    nc.to_json_bytes = patched


# ---------------------------------------------------------------- kernel build
def _rap(t, off, pairs):
    """Raw AP over pool tile t (full 128 partitions) with free dims pairs."""
    return AP(t.tensor, int(t.offset) + off, [list(t.ap[0])] + [list(p) for p in pairs])


def _delta(d):
    return 26 * d[0] + d[1]


def _emit_shift_group(nc, dst, src, src_slot0, dlist, m_sb, scalar, rt=None):
    """dst[o] = (sh_{dlist[o]}(src[slot o+src_slot0]) * scalar) * m, for the
    three plane-pairs. If scalar is None, plain mult by m (RT build)."""
    mul = mybir.AluOpType.mult
    for oa, ob in PAIRS:
        da, db = _delta(dlist[oa]), _delta(dlist[ob])
        step = (ob - oa) * SLOT + (db - da)
        in0 = _rap(src, (oa + src_slot0) * SLOT + DOF + da, [[step, 2], [1, DN]])
        out = _rap(dst, oa * SLOT + DOF, [[(ob - oa) * SLOT, 2], [1, DN]])
        m_b = _rap(m_sb, DOF, [[0, 2], [1, DN]])
        if scalar is None:
            nc.vector.tensor_tensor(out=out, in0=in0, in1=m_b, op=mul)
        else:
            nc.vector.scalar_tensor_tensor(
                out=out, in0=in0, scalar=scalar, in1=m_b, op0=mul, op1=mul)


def build_nc(n_iter):
    nc = bass.Bass()
    _install_compat(nc)
    mx = mybir.AluOpType.max
    mul = mybir.AluOpType.mult
    add = mybir.AluOpType.add

    goals_d = nc.declare_dram_parameter("goals", [128, 6, SLOT], DTYPE, isOutput=False)
    m_d = nc.declare_dram_parameter("m", [128, SLOT], DTYPE, isOutput=False)
    q0_d = nc.declare_dram_parameter("q0", [128, 6, SLOT], DTYPE, isOutput=True)
    q1_d = nc.declare_dram_parameter("q1", [128, 6, SLOT], DTYPE, isOutput=True)
    w_d = nc.declare_dram_parameter("w", [128, 6, SLOT], DTYPE, isOutput=True)

    with TileContext(nc) as tc:
        with tc.tile_pool(name="p", bufs=1) as pool:
            m_sb = pool.tile([128, SLOT], DTYPE)
            mg_sb = pool.tile([128, SLOT], DTYPE)     # gamma * m
            gme = pool.tile([128, 6, SLOT], DTYPE)
            v = pool.tile([128, 6, SLOT], DTYPE)      # holds gamma*V (prescaled)
            t0 = pool.tile([128, 6, SLOT], DTYPE)
            t1 = pool.tile([128, 6, SLOT], DTYPE)
            we = pool.tile([128, 8, SLOT], DTYPE)     # W slots 1..6, dups 0/7

            nc.gpsimd.memset(v[:], 0.0)
            nc.gpsimd.memset(we[:], 0.0)
            nc.sync.dma_start(out=m_sb[:], in_=m_d[:])
            goals_sb = pool.tile([128, 6, SLOT], DTYPE)
            nc.sync.dma_start(out=goals_sb[:], in_=goals_d[:])

            # gm = goals * m (full slots incl halo rows; host fills halos)
            m_b6 = _rap(m_sb, 0, [[0, 6], [1, SLOT]])
            nc.vector.tensor_tensor(out=gme[:], in0=goals_sb[:], in1=m_b6, op=mul)

            dv = (slice(None), slice(None), slice(DOF, DOF + DN))  # data view

            def halo_we():
                # half1 top halo <- half0 grid row 18 (buffer row 19)
                nc.sync.dma_start(out=we[64:128, 1:7, 0:26],
                                  in_=we[0:64, 1:7, 19 * 26:20 * 26])
                # half0 bottom halo <- half1 grid row 19 (buffer row 1)
                nc.sync.dma_start(out=we[0:64, 1:7, 20 * 26:21 * 26],
                                  in_=we[64:128, 1:7, 26:52])

            def w_build():
                # v holds gamma*V already, so W = v + gm. Boundary rows
                # (1, 19) first so the halo DMAs overlap the interior add.
                # rows 1 and 19 of every slot: [[546,6],[468,2],[1,26]]
                wa = _rap(we, SLOT + 26, [[SLOT, 6], [468, 2], [1, 26]])
                ga = _rap(v, 26, [[SLOT, 6], [468, 2], [1, 26]])
                ma = _rap(gme, 26, [[SLOT, 6], [468, 2], [1, 26]])
                nc.vector.tensor_tensor(out=wa, in0=ga, in1=ma, op=add)
                halo_we()
                # interior rows 2..18 (contiguous 442) of every slot
                wb = _rap(we, SLOT + 52, [[SLOT, 6], [1, 442]])
                gb = _rap(v, 52, [[SLOT, 6], [1, 442]])
                mb = _rap(gme, 52, [[SLOT, 6], [1, 442]])
                nc.vector.tensor_tensor(out=wb, in0=gb, in1=mb, op=add)

            def dups():
                nc.scalar.copy(out=we[:, 0, DOF:DOF + DN],
                               in_=we[:, 6, DOF:DOF + DN])
                nc.scalar.copy(out=we[:, 7, DOF:DOF + DN],
                               in_=we[:, 1, DOF:DOF + DN])

            def t_build():
                _emit_shift_group(nc, t0, we, 1, D0, m_sb, None)
                _emit_shift_group(nc, t1, we, 1, D1, m_sb, None)

            def x_build():
                # X[o] = max(sh_{+d0(o)}(W[o]), sh_{-d0(o)}(W[o])) -> t0
                # (D1 = -D0, so max(T0,T1) = m * X)
                for oa, ob in PAIRS:
                    da, db = _delta(D0[oa]), _delta(D0[ob])
                    in0 = _rap(we, (oa + 1) * SLOT + DOF + da,
                               [[(ob - oa) * SLOT + (db - da), 2], [1, DN]])
                    in1 = _rap(we, (oa + 1) * SLOT + DOF - da,
                               [[(ob - oa) * SLOT - (db - da), 2], [1, DN]])
                    out = _rap(t0, oa * SLOT + DOF, [[(ob - oa) * SLOT, 2], [1, DN]])
                    nc.vector.tensor_tensor(out=out, in0=in0, in1=in1, op=mx)

            # mg = gamma*m once; the loop's final mask-mult then yields
            # gamma*V directly, eliminating a per-iter tensor_scalar.
            nc.vector.tensor_scalar_mul(out=mg_sb[:], in0=m_sb[:], scalar1=GAMMA)
            mg_b6d = _rap(mg_sb, DOF, [[0, 6], [1, DN]])

            for it in range(n_iter):
                w_build()
                dups()
                x_build()
                # M2 = max(W', W''); Z = max(X, M2); v = (gamma*m) * Z
                nc.vector.tensor_tensor(
                    out=t1[dv], in0=we[:, 2:8, DOF:DOF + DN],
                    in1=we[:, 0:6, DOF:DOF + DN], op=mx)
                nc.vector.tensor_tensor(out=t0[dv], in0=t0[dv], in1=t1[dv], op=mx)
                nc.vector.tensor_tensor(out=v[dv], in0=t0[dv], in1=mg_b6d, op=mul)

            # final partial: Q0 = m*sh0(W), Q1 = m*sh1(W), q2/q3 = W views
            w_build()
            t_build()

            nc.sync.dma_start(out=q0_d[:], in_=t0[:])
            nc.sync.dma_start(out=q1_d[:], in_=t1[:])
            nc.sync.dma_start(out=w_d[:], in_=we[:, 1:7])
    return nc


_NC_CACHE = {}


def _get_nc(n_iter):
    if n_iter not in _NC_CACHE:
        _NC_CACHE[n_iter] = build_nc(n_iter)
    return _NC_CACHE[n_iter]


# ---------------------------------------------------------------- host side
def _to_padded_axial(x):
    out = np.zeros(x.shape[:-2] + (PR, PC), np.float32)
    out[..., _ROW, _COL] = x
    return out


def kernel(offset_input_goals, offset_current_state, offset_obstacles,
           num_iterations):
    global LAST_RESULT
    goals = np.asarray(offset_input_goals, np.float32)
    state = np.asarray(offset_current_state)
    obst = np.asarray(offset_obstacles, np.float32)
    n_iter = int(num_iterations)
    B = goals.shape[0]
    assert B == N_CORES * BPC and n_iter >= 1

    goals_ax = _to_padded_axial(goals)                     # [B,6,40,26]
    mask = _to_padded_axial(np.ones((E, E), np.float32))
    m_full = (1.0 - _to_padded_axial(obst)) * mask         # [B,40,26]

    def split(x):  # [B, ..., 40, 26] -> [B, 2, ..., 546]
        h0 = x[..., 0:21, :].reshape(x.shape[:-2] + (SLOT,))
        h1 = x[..., 19:40, :].reshape(x.shape[:-2] + (SLOT,))
        return h0, h1

    g0, g1 = split(goals_ax)
    m0, m1 = split(m_full)

    in_maps = []
    for c in range(N_CORES):
        s = slice(c * BPC, (c + 1) * BPC)
        in_maps.append({
            "goals": np.concatenate([g0[s], g1[s]], 0).astype(NP_DT),
            "m": np.concatenate([m0[s], m1[s]], 0).astype(NP_DT),
        })

    nc = _get_nc(n_iter)
    res = run_bass_kernel_spmd(nc, in_maps, core_ids=list(range(N_CORES)),
                               trace=TRACE)
    LAST_RESULT = res

    out = np.zeros((B, 4), np.float32)
    alpha = state[:, 0].astype(np.int64)
    uu = (state[:, 1] - state[:, 2] // 2 + ADD).astype(np.int64)
    vv = state[:, 2].astype(np.int64)
    rot = (alpha + 1) % 6
    h = (uu > 18).astype(np.int64)
    r = np.where(h == 0, uu + 1, uu - 18)
    idx = r * 26 + vv
    for c in range(N_CORES):
        rr = res.results[c]
        q0 = np.asarray(rr["q0"], np.float32)
        q1 = np.asarray(rr["q1"], np.float32)
        w = np.asarray(rr["w"], np.float32)
        bs = np.arange(c * BPC, (c + 1) * BPC)
        p = h[bs] * 64 + np.arange(BPC)
        out[bs, 0] = q0[p, rot[bs], idx[bs]]
        out[bs, 1] = q1[p, rot[bs], idx[bs]]
        out[bs, 2] = w[p, (rot[bs] + 1) % 6, idx[bs]]
        out[bs, 3] = w[p, (rot[bs] + 5) % 6, idx[bs]]
    return out


# revision 15
# speedup vs baseline: 1.3460x; 1.0697x over previous
"""Trainium2 Bass kernel for the CerealBar VIN problem.

Self-contained: hardcodes shapes B=512, E=25, 6 orientations, gamma=0.9,
8-core batch sharding (64 samples/core).

Math (derived from the reference, validated bit-exact in numpy):
  The grouped 3x3 conv is a set of one-hot spatial shifts. With
  m = (1-obstacles_axial) * axial_mask (0/1), gm[o] = goals_axial[o]*m:
    RT0[o] = m * sh_{d0(o)}(gm[o]);  RT1[o] = m * sh_{d1(o)}(gm[o])
    v0[o]  = max(RT0[o], RT1[o], gm[(o+1)%6], gm[(o+5)%6])
    repeat n-1 times:
      T0[o] = m * g * sh_{d0(o)}(v[o]);  T1[o] = m * g * sh_{d1(o)}(v[o])
      W[o]  = g*v[o] + gm[o]
      v[o]  = max(T0[o]+RT0[o], T1[o]+RT1[o], W[(o+1)%6], W[(o+5)%6])
    Q0 = T0+RT0, Q1 = T1+RT1, W = g*v+gm   (from final v)
    out[b] = [Q0[rot,uu,vv], Q1[rot,uu,vv], W[(rot+1)%6][uu,vv],
              W[(rot+5)%6][uu,vv]],  rot=(alpha+1)%6, uu=u-v//2+12

Device layout: partition p = h*64 + i -> sample i of the core, row-half h.
Each (orientation, half) plane = 21 rows x 26 cols flat (546): row 0 top
halo, rows 1..19 data, row 20 bottom halo, col 25 zero pad. half0 data =
grid rows 0..18, half1 = grid rows 19..37 (37 is a fake all-masked row).
Shifts are free-dim AP offsets (d = 26*dy+dx); the row-split halo rows are
refreshed once per iteration by two cross-partition SBUF-SBUF DMAs.
"""
import json
import sys

sys.path.insert(0, "/opt/trn_rl_repo")

import numpy as np

import concourse.bass as bass
import concourse.mybir as mybir
from concourse.ap import AP
from concourse.bass_utils import run_bass_kernel_spmd
from concourse.tile import TileContext

E = 25
ADD = 12
GAMMA = 0.9
PR = 40          # padded full-grid rows (grid rows -1..38 at idx r+1)
PC = 26
SLOT = 546       # 21 * 26 per half-plane
DOF = 26         # data offset within a slot (row 1)
DN = 494         # data elems (rows 1..19)
N_CORES = 8
BPC = 64         # samples per core

# shifts (dy, dx): out[y, x] = in[y+dy, x+dx]
D0 = [(0, 1), (1, 0), (1, -1), (0, -1), (-1, 0), (-1, 1)]
D1 = [(0, -1), (-1, 0), (-1, 1), (0, 1), (1, 0), (1, -1)]
PAIRS = [(0, 3), (1, 2), (4, 5)]  # (0,3) first: dy=0, no halo-row dep

import os as _os

if _os.environ.get("KDT", "fp16") == "fp16":
    DTYPE = mybir.dt.float16
    NP_DT = np.float16
else:
    DTYPE = mybir.dt.float32
    NP_DT = np.float32

TRACE = False
LAST_RESULT = None

_u = np.arange(E)[:, None]
_v = np.arange(E)[None, :]
_ROW = (_u - _v // 2 + ADD) + 1
_COL = np.broadcast_to(_v, (E, E))


# ---------------------------------------------------------------- BIR fixups
def _split_multi_waits(bir):
    """The installed walrus rejects >1 sync wait per instruction; hoist
    extras onto single-wait NoOps inserted before it on the same engine."""
    for fn in bir.get("functions", []):
        for blk in fn.get("blocks", []):
            out = []
            for ins in blk.get("instructions", []):
                si = ins.get("sync_info")
                waits = (si or {}).get("on_wait") or []
                if len(waits) > 1:
                    for k, w in enumerate(waits[:-1]):
                        out.append({
                            "debug": ins.get("debug", 0),
                            "engine": ins["engine"],
                            "ins": [], "outs": [],
                            "name": f"{ins['name']}_w{k}",
                            "opcode": "NoOp",
                            "sync_info": {"on_wait": [w], "on_update": []},
                            "text_hint": "split_wait",
                        })
                    si["on_wait"] = [waits[-1]]
                out.append(ins)
            blk["instructions"] = out
    return bir


def _install_compat(nc):
    orig = nc.to_json_bytes

    def patched():
        return json.dumps(_split_multi_waits(json.loads(orig()))).encode()

    nc.to_json_bytes = patched


# ---------------------------------------------------------------- kernel build
def _rap(t, off, pairs):
    """Raw AP over pool tile t (full 128 partitions) with free dims pairs."""
    return AP(t.tensor, int(t.offset) + off, [list(t.ap[0])] + [list(p) for p in pairs])


def _delta(d):
    return 26 * d[0] + d[1]


def _emit_shift_group(nc, dst, src, src_slot0, dlist, m_sb, scalar, rt=None):
    """dst[o] = (sh_{dlist[o]}(src[slot o+src_slot0]) * scalar) * m, for the
    three plane-pairs. If scalar is None, plain mult by m (RT build)."""
    mul = mybir.AluOpType.mult
    for oa, ob in PAIRS:
        da, db = _delta(dlist[oa]), _delta(dlist[ob])
        step = (ob - oa) * SLOT + (db - da)
        in0 = _rap(src, (oa + src_slot0) * SLOT + DOF + da, [[step, 2], [1, DN]])
        out = _rap(dst, oa * SLOT + DOF, [[(ob - oa) * SLOT, 2], [1, DN]])
        m_b = _rap(m_sb, DOF, [[0, 2], [1, DN]])
        if scalar is None:
            nc.vector.tensor_tensor(out=out, in0=in0, in1=m_b, op=mul)
        else:
            nc.vector.scalar_tensor_tensor(
                out=out, in0=in0, scalar=scalar, in1=m_b, op0=mul, op1=mul)


def build_nc(n_iter):
    nc = bass.Bass()
    _install_compat(nc)
    mx = mybir.AluOpType.max
    mul = mybir.AluOpType.mult
    add = mybir.AluOpType.add

    goals_d = nc.declare_dram_parameter("goals", [128, 6, SLOT], DTYPE, isOutput=False)
    m_d = nc.declare_dram_parameter("m", [128, SLOT], DTYPE, isOutput=False)
    q0_d = nc.declare_dram_parameter("q0", [128, 6, SLOT], DTYPE, isOutput=True)
    q1_d = nc.declare_dram_parameter("q1", [128, 6, SLOT], DTYPE, isOutput=True)
    w_d = nc.declare_dram_parameter("w", [128, 6, SLOT], DTYPE, isOutput=True)

    with TileContext(nc) as tc:
        with tc.tile_pool(name="p", bufs=1) as pool:
            m_sb = pool.tile([128, SLOT], DTYPE)
            mg_sb = pool.tile([128, SLOT], DTYPE)     # gamma * m
            gme = pool.tile([128, 6, SLOT], DTYPE)
            v = pool.tile([128, 6, SLOT], DTYPE)      # holds gamma*V (prescaled)
            t0 = pool.tile([128, 6, SLOT], DTYPE)
            t1 = pool.tile([128, 6, SLOT], DTYPE)
            we = pool.tile([128, 8, SLOT], DTYPE)     # W slots 1..6, dups 0/7

            nc.gpsimd.memset(v[:], 0.0)
            nc.gpsimd.memset(we[:], 0.0)
            nc.sync.dma_start(out=m_sb[:], in_=m_d[:])
            goals_sb = pool.tile([128, 6, SLOT], DTYPE)
            nc.sync.dma_start(out=goals_sb[:], in_=goals_d[:])

            # gm = goals * m (full slots incl halo rows; host fills halos)
            m_b6 = _rap(m_sb, 0, [[0, 6], [1, SLOT]])
            nc.vector.tensor_tensor(out=gme[:], in0=goals_sb[:], in1=m_b6, op=mul)

            dv = (slice(None), slice(None), slice(DOF, DOF + DN))  # data view

            def halo_we():
                # half1 top halo <- half0 grid row 18 (buffer row 19)
                nc.sync.dma_start(out=we[64:128, 1:7, 0:26],
                                  in_=we[0:64, 1:7, 19 * 26:20 * 26])
                # half0 bottom halo <- half1 grid row 19 (buffer row 1)
                nc.sync.dma_start(out=we[0:64, 1:7, 20 * 26:21 * 26],
                                  in_=we[64:128, 1:7, 26:52])

            def w_build():
                # v holds gamma*V already, so W = v + gm. Boundary rows
                # (1, 19) first so the halo DMAs overlap the interior add.
                # rows 1 and 19 of every slot: [[546,6],[468,2],[1,26]]
                wa = _rap(we, SLOT + 26, [[SLOT, 6], [468, 2], [1, 26]])
                ga = _rap(v, 26, [[SLOT, 6], [468, 2], [1, 26]])
                ma = _rap(gme, 26, [[SLOT, 6], [468, 2], [1, 26]])
                nc.vector.tensor_tensor(out=wa, in0=ga, in1=ma, op=add)
                halo_we()
                # interior rows 2..18 (contiguous 442) of every slot
                wb = _rap(we, SLOT + 52, [[SLOT, 6], [1, 442]])
                gb = _rap(v, 52, [[SLOT, 6], [1, 442]])
                mb = _rap(gme, 52, [[SLOT, 6], [1, 442]])
                nc.vector.tensor_tensor(out=wb, in0=gb, in1=mb, op=add)

            def dups():
                # DVE fp16 copies hit the 4x mode (~190ns); placed between
                # the interior W-add and the halo-dependent X ops they also
                # widen the DMA-overlap window on the DVE queue.
                nc.vector.tensor_copy(out=we[:, 0, DOF:DOF + DN],
                                      in_=we[:, 6, DOF:DOF + DN])
                nc.vector.tensor_copy(out=we[:, 7, DOF:DOF + DN],
                                      in_=we[:, 1, DOF:DOF + DN])

            def t_build():
                _emit_shift_group(nc, t0, we, 1, D0, m_sb, None)
                _emit_shift_group(nc, t1, we, 1, D1, m_sb, None)

            def x_build():
                # X[o] = max(sh_{+d0(o)}(W[o]), sh_{-d0(o)}(W[o])) -> t0
                # (D1 = -D0, so max(T0,T1) = m * X)
                for oa, ob in PAIRS:
                    da, db = _delta(D0[oa]), _delta(D0[ob])
                    in0 = _rap(we, (oa + 1) * SLOT + DOF + da,
                               [[(ob - oa) * SLOT + (db - da), 2], [1, DN]])
                    in1 = _rap(we, (oa + 1) * SLOT + DOF - da,
                               [[(ob - oa) * SLOT - (db - da), 2], [1, DN]])
                    out = _rap(t0, oa * SLOT + DOF, [[(ob - oa) * SLOT, 2], [1, DN]])
                    nc.vector.tensor_tensor(out=out, in0=in0, in1=in1, op=mx)

            # mg = gamma*m once; the loop's final mask-mult then yields
            # gamma*V directly, eliminating a per-iter tensor_scalar.
            nc.vector.tensor_scalar_mul(out=mg_sb[:], in0=m_sb[:], scalar1=GAMMA)
            mg_b6d = _rap(mg_sb, DOF, [[0, 6], [1, DN]])

            for it in range(n_iter):
                w_build()
                dups()
                x_build()
                # M2 = max(W', W''); Z = max(X, M2); v = (gamma*m) * Z
                nc.vector.tensor_tensor(
                    out=t1[dv], in0=we[:, 2:8, DOF:DOF + DN],
                    in1=we[:, 0:6, DOF:DOF + DN], op=mx)
                nc.vector.tensor_tensor(out=t0[dv], in0=t0[dv], in1=t1[dv], op=mx)
                nc.vector.tensor_tensor(out=v[dv], in0=t0[dv], in1=mg_b6d, op=mul)

            # final partial: Q0 = m*sh0(W), Q1 = m*sh1(W), q2/q3 = W views
            w_build()
            t_build()

            nc.sync.dma_start(out=q0_d[:], in_=t0[:])
            nc.sync.dma_start(out=q1_d[:], in_=t1[:])
            nc.sync.dma_start(out=w_d[:], in_=we[:, 1:7])
    return nc


_NC_CACHE = {}


def _get_nc(n_iter):
    if n_iter not in _NC_CACHE:
        _NC_CACHE[n_iter] = build_nc(n_iter)
    return _NC_CACHE[n_iter]


# ---------------------------------------------------------------- host side
def _to_padded_axial(x):
    out = np.zeros(x.shape[:-2] + (PR, PC), np.float32)
    out[..., _ROW, _COL] = x
    return out


def kernel(offset_input_goals, offset_current_state, offset_obstacles,
           num_iterations):
    global LAST_RESULT
    goals = np.asarray(offset_input_goals, np.float32)
    state = np.asarray(offset_current_state)
    obst = np.asarray(offset_obstacles, np.float32)
    n_iter = int(num_iterations)
    B = goals.shape[0]
    assert B == N_CORES * BPC and n_iter >= 1

    goals_ax = _to_padded_axial(goals)                     # [B,6,40,26]
    mask = _to_padded_axial(np.ones((E, E), np.float32))
    m_full = (1.0 - _to_padded_axial(obst)) * mask         # [B,40,26]

    def split(x):  # [B, ..., 40, 26] -> [B, 2, ..., 546]
        h0 = x[..., 0:21, :].reshape(x.shape[:-2] + (SLOT,))
        h1 = x[..., 19:40, :].reshape(x.shape[:-2] + (SLOT,))
        return h0, h1

    g0, g1 = split(goals_ax)
    m0, m1 = split(m_full)

    in_maps = []
    for c in range(N_CORES):
        s = slice(c * BPC, (c + 1) * BPC)
        in_maps.append({
            "goals": np.concatenate([g0[s], g1[s]], 0).astype(NP_DT),
            "m": np.concatenate([m0[s], m1[s]], 0).astype(NP_DT),
        })

    nc = _get_nc(n_iter)
    res = run_bass_kernel_spmd(nc, in_maps, core_ids=list(range(N_CORES)),
                               trace=TRACE)
    LAST_RESULT = res

    out = np.zeros((B, 4), np.float32)
    alpha = state[:, 0].astype(np.int64)
    uu = (state[:, 1] - state[:, 2] // 2 + ADD).astype(np.int64)
    vv = state[:, 2].astype(np.int64)
    rot = (alpha + 1) % 6
    h = (uu > 18).astype(np.int64)
    r = np.where(h == 0, uu + 1, uu - 18)
    idx = r * 26 + vv
    for c in range(N_CORES):
        rr = res.results[c]
        q0 = np.asarray(rr["q0"], np.float32)
        q1 = np.asarray(rr["q1"], np.float32)
        w = np.asarray(rr["w"], np.float32)
        bs = np.arange(c * BPC, (c + 1) * BPC)
        p = h[bs] * 64 + np.arange(BPC)
        out[bs, 0] = q0[p, rot[bs], idx[bs]]
        out[bs, 1] = q1[p, rot[bs], idx[bs]]
        out[bs, 2] = w[p, (rot[bs] + 1) % 6, idx[bs]]
        out[bs, 3] = w[p, (rot[bs] + 5) % 6, idx[bs]]
    return out


# revision 17
# speedup vs baseline: 1.3518x; 1.0043x over previous
"""Trainium2 Bass kernel for the CerealBar VIN problem.

Self-contained: hardcodes shapes B=512, E=25, 6 orientations, gamma=0.9,
8-core batch sharding (64 samples/core).

Math (derived from the reference, validated bit-exact in numpy):
  The grouped 3x3 conv is a set of one-hot spatial shifts. With
  m = (1-obstacles_axial) * axial_mask (0/1), gm[o] = goals_axial[o]*m:
    RT0[o] = m * sh_{d0(o)}(gm[o]);  RT1[o] = m * sh_{d1(o)}(gm[o])
    v0[o]  = max(RT0[o], RT1[o], gm[(o+1)%6], gm[(o+5)%6])
    repeat n-1 times:
      T0[o] = m * g * sh_{d0(o)}(v[o]);  T1[o] = m * g * sh_{d1(o)}(v[o])
      W[o]  = g*v[o] + gm[o]
      v[o]  = max(T0[o]+RT0[o], T1[o]+RT1[o], W[(o+1)%6], W[(o+5)%6])
    Q0 = T0+RT0, Q1 = T1+RT1, W = g*v+gm   (from final v)
    out[b] = [Q0[rot,uu,vv], Q1[rot,uu,vv], W[(rot+1)%6][uu,vv],
              W[(rot+5)%6][uu,vv]],  rot=(alpha+1)%6, uu=u-v//2+12

Device layout: partition p = h*64 + i -> sample i of the core, row-half h.
Each (orientation, half) plane = 21 rows x 26 cols flat (546): row 0 top
halo, rows 1..19 data, row 20 bottom halo, col 25 zero pad. half0 data =
grid rows 0..18, half1 = grid rows 19..37 (37 is a fake all-masked row).
Shifts are free-dim AP offsets (d = 26*dy+dx); the row-split halo rows are
refreshed once per iteration by two cross-partition SBUF-SBUF DMAs.
"""
import json
import sys

sys.path.insert(0, "/opt/trn_rl_repo")

import numpy as np

import concourse.bass as bass
import concourse.mybir as mybir
from concourse.ap import AP
from concourse.bass_utils import run_bass_kernel_spmd
from concourse.tile import TileContext

E = 25
ADD = 12
GAMMA = 0.9
PR = 40          # padded full-grid rows (grid rows -1..38 at idx r+1)
PC = 26
SLOT = 546       # 21 * 26 per half-plane
DOF = 26         # data offset within a slot (row 1)
DN = 494         # data elems (rows 1..19)
N_CORES = 8
BPC = 64         # samples per core

# shifts (dy, dx): out[y, x] = in[y+dy, x+dx]
D0 = [(0, 1), (1, 0), (1, -1), (0, -1), (-1, 0), (-1, 1)]
D1 = [(0, -1), (-1, 0), (-1, 1), (0, 1), (1, 0), (1, -1)]
PAIRS = [(0, 3), (1, 2), (4, 5)]  # (0,3) first: dy=0, no halo-row dep

import os as _os

if _os.environ.get("KDT", "fp16") == "fp16":
    DTYPE = mybir.dt.float16
    NP_DT = np.float16
else:
    DTYPE = mybir.dt.float32
    NP_DT = np.float32

TRACE = False
LAST_RESULT = None

_u = np.arange(E)[:, None]
_v = np.arange(E)[None, :]
_ROW = (_u - _v // 2 + ADD) + 1
_COL = np.broadcast_to(_v, (E, E))


# ---------------------------------------------------------------- BIR fixups
def _split_multi_waits(bir):
    """The installed walrus rejects >1 sync wait per instruction; hoist
    extras onto single-wait NoOps inserted before it on the same engine."""
    for fn in bir.get("functions", []):
        for blk in fn.get("blocks", []):
            out = []
            for ins in blk.get("instructions", []):
                si = ins.get("sync_info")
                waits = (si or {}).get("on_wait") or []
                if len(waits) > 1:
                    for k, w in enumerate(waits[:-1]):
                        out.append({
                            "debug": ins.get("debug", 0),
                            "engine": ins["engine"],
                            "ins": [], "outs": [],
                            "name": f"{ins['name']}_w{k}",
                            "opcode": "NoOp",
                            "sync_info": {"on_wait": [w], "on_update": []},
                            "text_hint": "split_wait",
                        })
                    si["on_wait"] = [waits[-1]]
                out.append(ins)
            blk["instructions"] = out
    return bir


def _install_compat(nc):
    orig = nc.to_json_bytes

    def patched():
        return json.dumps(_split_multi_waits(json.loads(orig()))).encode()

    nc.to_json_bytes = patched


# ---------------------------------------------------------------- kernel build
def _rap(t, off, pairs):
    """Raw AP over pool tile t (full 128 partitions) with free dims pairs."""
    return AP(t.tensor, int(t.offset) + off, [list(t.ap[0])] + [list(p) for p in pairs])


def _delta(d):
    return 26 * d[0] + d[1]


def _emit_shift_group(nc, dst, src, src_slot0, dlist, m_sb, scalar, rt=None):
    """dst[o] = (sh_{dlist[o]}(src[slot o+src_slot0]) * scalar) * m, for the
    three plane-pairs. If scalar is None, plain mult by m (RT build)."""
    mul = mybir.AluOpType.mult
    for oa, ob in PAIRS:
        da, db = _delta(dlist[oa]), _delta(dlist[ob])
        step = (ob - oa) * SLOT + (db - da)
        in0 = _rap(src, (oa + src_slot0) * SLOT + DOF + da, [[step, 2], [1, DN]])
        out = _rap(dst, oa * SLOT + DOF, [[(ob - oa) * SLOT, 2], [1, DN]])
        m_b = _rap(m_sb, DOF, [[0, 2], [1, DN]])
        if scalar is None:
            nc.vector.tensor_tensor(out=out, in0=in0, in1=m_b, op=mul)
        else:
            nc.vector.scalar_tensor_tensor(
                out=out, in0=in0, scalar=scalar, in1=m_b, op0=mul, op1=mul)


def build_nc(n_iter):
    nc = bass.Bass()
    _install_compat(nc)
    mx = mybir.AluOpType.max
    mul = mybir.AluOpType.mult
    add = mybir.AluOpType.add

    goals_d = nc.declare_dram_parameter("goals", [128, 6, SLOT], DTYPE, isOutput=False)
    m_d = nc.declare_dram_parameter("m", [128, SLOT], DTYPE, isOutput=False)
    q0_d = nc.declare_dram_parameter("q0", [128, 6, SLOT], DTYPE, isOutput=True)
    q1_d = nc.declare_dram_parameter("q1", [128, 6, SLOT], DTYPE, isOutput=True)
    w_d = nc.declare_dram_parameter("w", [128, 6, SLOT], DTYPE, isOutput=True)

    with TileContext(nc) as tc:
        with tc.tile_pool(name="p", bufs=1) as pool:
            m_sb = pool.tile([128, SLOT], DTYPE)
            mg_sb = pool.tile([128, SLOT], DTYPE)     # gamma * m
            gme = pool.tile([128, 6, SLOT], DTYPE)
            v = pool.tile([128, 6, SLOT], DTYPE)      # holds gamma*V (prescaled)
            t0 = pool.tile([128, 6, SLOT], DTYPE)
            t1 = pool.tile([128, 6, SLOT], DTYPE)
            we = pool.tile([128, 8, SLOT], DTYPE)     # W slots 1..6, dups 0/7

            nc.gpsimd.memset(v[:], 0.0)
            nc.gpsimd.memset(we[:], 0.0)
            nc.sync.dma_start(out=m_sb[:], in_=m_d[:])
            goals_sb = pool.tile([128, 6, SLOT], DTYPE)
            # chunked goals DMA so the gm build overlaps the transfer
            nc.sync.dma_start(out=goals_sb[:, 0:3], in_=goals_d[:, 0:3])
            nc.sync.dma_start(out=goals_sb[:, 3:6], in_=goals_d[:, 3:6])

            # gm = goals * m (full slots incl halo rows; host fills halos)
            m_b3 = _rap(m_sb, 0, [[0, 3], [1, SLOT]])
            nc.vector.tensor_tensor(out=gme[:, 0:3], in0=goals_sb[:, 0:3],
                                    in1=m_b3, op=mul)
            nc.vector.tensor_tensor(out=gme[:, 3:6], in0=goals_sb[:, 3:6],
                                    in1=m_b3, op=mul)

            dv = (slice(None), slice(None), slice(DOF, DOF + DN))  # data view

            def halo_we():
                # half1 top halo <- half0 grid row 18 (buffer row 19)
                nc.sync.dma_start(out=we[64:128, 1:7, 0:26],
                                  in_=we[0:64, 1:7, 19 * 26:20 * 26])
                # half0 bottom halo <- half1 grid row 19 (buffer row 1)
                nc.sync.dma_start(out=we[0:64, 1:7, 20 * 26:21 * 26],
                                  in_=we[64:128, 1:7, 26:52])

            def w_build():
                # v holds gamma*V already, so W = v + gm. Boundary rows
                # (1, 19) first so the halo DMAs overlap the interior add.
                # rows 1 and 19 of every slot: [[546,6],[468,2],[1,26]]
                wa = _rap(we, SLOT + 26, [[SLOT, 6], [468, 2], [1, 26]])
                ga = _rap(v, 26, [[SLOT, 6], [468, 2], [1, 26]])
                ma = _rap(gme, 26, [[SLOT, 6], [468, 2], [1, 26]])
                nc.vector.tensor_tensor(out=wa, in0=ga, in1=ma, op=add)
                halo_we()
                # interior rows 2..18 (contiguous 442) of every slot
                wb = _rap(we, SLOT + 52, [[SLOT, 6], [1, 442]])
                gb = _rap(v, 52, [[SLOT, 6], [1, 442]])
                mb = _rap(gme, 52, [[SLOT, 6], [1, 442]])
                nc.vector.tensor_tensor(out=wb, in0=gb, in1=mb, op=add)

            def dups():
                # DVE fp16 copies hit the 4x mode (~190ns); placed between
                # the interior W-add and the halo-dependent X ops they also
                # widen the DMA-overlap window on the DVE queue.
                nc.vector.tensor_copy(out=we[:, 0, DOF:DOF + DN],
                                      in_=we[:, 6, DOF:DOF + DN])
                nc.vector.tensor_copy(out=we[:, 7, DOF:DOF + DN],
                                      in_=we[:, 1, DOF:DOF + DN])

            def t_build():
                _emit_shift_group(nc, t0, we, 1, D0, m_sb, None)
                _emit_shift_group(nc, t1, we, 1, D1, m_sb, None)

            def x_build():
                # X[o] = max(sh_{+d0(o)}(W[o]), sh_{-d0(o)}(W[o])) -> t0
                # (D1 = -D0, so max(T0,T1) = m * X)
                for oa, ob in PAIRS:
                    da, db = _delta(D0[oa]), _delta(D0[ob])
                    in0 = _rap(we, (oa + 1) * SLOT + DOF + da,
                               [[(ob - oa) * SLOT + (db - da), 2], [1, DN]])
                    in1 = _rap(we, (oa + 1) * SLOT + DOF - da,
                               [[(ob - oa) * SLOT - (db - da), 2], [1, DN]])
                    out = _rap(t0, oa * SLOT + DOF, [[(ob - oa) * SLOT, 2], [1, DN]])
                    nc.vector.tensor_tensor(out=out, in0=in0, in1=in1, op=mx)

            # mg = gamma*m once; the loop's final mask-mult then yields
            # gamma*V directly, eliminating a per-iter tensor_scalar.
            nc.vector.tensor_scalar_mul(out=mg_sb[:], in0=m_sb[:], scalar1=GAMMA)
            mg_b6d = _rap(mg_sb, DOF, [[0, 6], [1, DN]])

            for it in range(n_iter):
                w_build()
                dups()
                x_build()
                # M2 = max(W', W''); Z = max(X, M2); v = (gamma*m) * Z
                nc.vector.tensor_tensor(
                    out=t1[dv], in0=we[:, 2:8, DOF:DOF + DN],
                    in1=we[:, 0:6, DOF:DOF + DN], op=mx)
                nc.vector.tensor_tensor(out=t0[dv], in0=t0[dv], in1=t1[dv], op=mx)
                nc.vector.tensor_tensor(out=v[dv], in0=t0[dv], in1=mg_b6d, op=mul)

            # final partial: Q0 = m*sh0(W), Q1 = m*sh1(W), q2/q3 = W views
            w_build()
            nc.sync.dma_start(out=w_d[:], in_=we[:, 1:7])   # overlaps t_build
            t_build()
            nc.sync.dma_start(out=q0_d[:], in_=t0[:])
            nc.sync.dma_start(out=q1_d[:], in_=t1[:])
    return nc


_NC_CACHE = {}


def _get_nc(n_iter):
    if n_iter not in _NC_CACHE:
        _NC_CACHE[n_iter] = build_nc(n_iter)
    return _NC_CACHE[n_iter]


# ---------------------------------------------------------------- host side
def _to_padded_axial(x):
    out = np.zeros(x.shape[:-2] + (PR, PC), np.float32)
    out[..., _ROW, _COL] = x
    return out


def kernel(offset_input_goals, offset_current_state, offset_obstacles,
           num_iterations):
    global LAST_RESULT
    goals = np.asarray(offset_input_goals, np.float32)
    state = np.asarray(offset_current_state)
    obst = np.asarray(offset_obstacles, np.float32)
    n_iter = int(num_iterations)
    B = goals.shape[0]
    assert B == N_CORES * BPC and n_iter >= 1

    goals_ax = _to_padded_axial(goals)                     # [B,6,40,26]
    mask = _to_padded_axial(np.ones((E, E), np.float32))
    m_full = (1.0 - _to_padded_axial(obst)) * mask         # [B,40,26]

    def split(x):  # [B, ..., 40, 26] -> [B, 2, ..., 546]
        h0 = x[..., 0:21, :].reshape(x.shape[:-2] + (SLOT,))
        h1 = x[..., 19:40, :].reshape(x.shape[:-2] + (SLOT,))
        return h0, h1

    g0, g1 = split(goals_ax)
    m0, m1 = split(m_full)

    in_maps = []
    for c in range(N_CORES):
        s = slice(c * BPC, (c + 1) * BPC)
        in_maps.append({
            "goals": np.concatenate([g0[s], g1[s]], 0).astype(NP_DT),
            "m": np.concatenate([m0[s], m1[s]], 0).astype(NP_DT),
        })

    nc = _get_nc(n_iter)
    res = run_bass_kernel_spmd(nc, in_maps, core_ids=list(range(N_CORES)),
                               trace=TRACE)
    LAST_RESULT = res

    out = np.zeros((B, 4), np.float32)
    alpha = state[:, 0].astype(np.int64)
    uu = (state[:, 1] - state[:, 2] // 2 + ADD).astype(np.int64)
    vv = state[:, 2].astype(np.int64)
    rot = (alpha + 1) % 6
    h = (uu > 18).astype(np.int64)
    r = np.where(h == 0, uu + 1, uu - 18)
    idx = r * 26 + vv
    for c in range(N_CORES):
        rr = res.results[c]
        q0 = np.asarray(rr["q0"], np.float32)
        q1 = np.asarray(rr["q1"], np.float32)
        w = np.asarray(rr["w"], np.float32)
        bs = np.arange(c * BPC, (c + 1) * BPC)
        p = h[bs] * 64 + np.arange(BPC)
        out[bs, 0] = q0[p, rot[bs], idx[bs]]
        out[bs, 1] = q1[p, rot[bs], idx[bs]]
        out[bs, 2] = w[p, (rot[bs] + 1) % 6, idx[bs]]
        out[bs, 3] = w[p, (rot[bs] + 5) % 6, idx[bs]]
    return out


# revision 19
# speedup vs baseline: 1.3827x; 1.0229x over previous
"""Trainium2 Bass kernel for the CerealBar VIN problem.

Self-contained: hardcodes shapes B=512, E=25, 6 orientations, gamma=0.9,
8-core batch sharding (64 samples/core).

Math (derived from the reference, validated bit-exact in numpy):
  The grouped 3x3 conv is a set of one-hot spatial shifts. With
  m = (1-obstacles_axial) * axial_mask (0/1), gm[o] = goals_axial[o]*m:
    RT0[o] = m * sh_{d0(o)}(gm[o]);  RT1[o] = m * sh_{d1(o)}(gm[o])
    v0[o]  = max(RT0[o], RT1[o], gm[(o+1)%6], gm[(o+5)%6])
    repeat n-1 times:
      T0[o] = m * g * sh_{d0(o)}(v[o]);  T1[o] = m * g * sh_{d1(o)}(v[o])
      W[o]  = g*v[o] + gm[o]
      v[o]  = max(T0[o]+RT0[o], T1[o]+RT1[o], W[(o+1)%6], W[(o+5)%6])
    Q0 = T0+RT0, Q1 = T1+RT1, W = g*v+gm   (from final v)
    out[b] = [Q0[rot,uu,vv], Q1[rot,uu,vv], W[(rot+1)%6][uu,vv],
              W[(rot+5)%6][uu,vv]],  rot=(alpha+1)%6, uu=u-v//2+12

Device layout: partition p = h*64 + i -> sample i of the core, row-half h.
Each (orientation, half) plane = 21 rows x 26 cols flat (546): row 0 top
halo, rows 1..19 data, row 20 bottom halo, col 25 zero pad. half0 data =
grid rows 0..18, half1 = grid rows 19..37 (37 is a fake all-masked row).
Shifts are free-dim AP offsets (d = 26*dy+dx); the row-split halo rows are
refreshed once per iteration by two cross-partition SBUF-SBUF DMAs.
"""
import json
import sys

sys.path.insert(0, "/opt/trn_rl_repo")

import numpy as np

import concourse.bass as bass
import concourse.mybir as mybir
from concourse.ap import AP
from concourse.bass_utils import run_bass_kernel_spmd
from concourse.tile import TileContext

E = 25
ADD = 12
GAMMA = 0.9
PR = 40          # padded full-grid rows (grid rows -1..38 at idx r+1)
PC = 26
SLOT = 546       # 21 * 26 per half-plane
DOF = 26         # data offset within a slot (row 1)
DN = 494         # data elems (rows 1..19)
N_CORES = 8
BPC = 64         # samples per core

# shifts (dy, dx): out[y, x] = in[y+dy, x+dx]
D0 = [(0, 1), (1, 0), (1, -1), (0, -1), (-1, 0), (-1, 1)]
D1 = [(0, -1), (-1, 0), (-1, 1), (0, 1), (1, 0), (1, -1)]
PAIRS = [(0, 3), (1, 2), (4, 5)]  # (0,3) first: dy=0, no halo-row dep

import os as _os

if _os.environ.get("KDT", "fp16") == "fp16":
    DTYPE = mybir.dt.float16
    NP_DT = np.float16
else:
    DTYPE = mybir.dt.float32
    NP_DT = np.float32

TRACE = False
LAST_RESULT = None

_u = np.arange(E)[:, None]
_v = np.arange(E)[None, :]
_ROW = (_u - _v // 2 + ADD) + 1
_COL = np.broadcast_to(_v, (E, E))


# ---------------------------------------------------------------- BIR fixups
def _split_multi_waits(bir):
    """The installed walrus rejects >1 sync wait per instruction; hoist
    extras onto single-wait NoOps inserted before it on the same engine."""
    for fn in bir.get("functions", []):
        for blk in fn.get("blocks", []):
            out = []
            for ins in blk.get("instructions", []):
                si = ins.get("sync_info")
                waits = (si or {}).get("on_wait") or []
                if len(waits) > 1:
                    for k, w in enumerate(waits[:-1]):
                        out.append({
                            "debug": ins.get("debug", 0),
                            "engine": ins["engine"],
                            "ins": [], "outs": [],
                            "name": f"{ins['name']}_w{k}",
                            "opcode": "NoOp",
                            "sync_info": {"on_wait": [w], "on_update": []},
                            "text_hint": "split_wait",
                        })
                    si["on_wait"] = [waits[-1]]
                out.append(ins)
            blk["instructions"] = out
    return bir


def _install_compat(nc):
    orig = nc.to_json_bytes

    def patched():
        return json.dumps(_split_multi_waits(json.loads(orig()))).encode()

    nc.to_json_bytes = patched


# ---------------------------------------------------------------- kernel build
def _rap(t, off, pairs):
    """Raw AP over pool tile t (full 128 partitions) with free dims pairs."""
    return AP(t.tensor, int(t.offset) + off, [list(t.ap[0])] + [list(p) for p in pairs])


def _delta(d):
    return 26 * d[0] + d[1]


def _emit_shift_group(nc, dst, src, src_slot0, dlist, m_sb, scalar, rt=None):
    """dst[o] = (sh_{dlist[o]}(src[slot o+src_slot0]) * scalar) * m, for the
    three plane-pairs. If scalar is None, plain mult by m (RT build)."""
    mul = mybir.AluOpType.mult
    for oa, ob in PAIRS:
        da, db = _delta(dlist[oa]), _delta(dlist[ob])
        step = (ob - oa) * SLOT + (db - da)
        in0 = _rap(src, (oa + src_slot0) * SLOT + DOF + da, [[step, 2], [1, DN]])
        out = _rap(dst, oa * SLOT + DOF, [[(ob - oa) * SLOT, 2], [1, DN]])
        m_b = _rap(m_sb, DOF, [[0, 2], [1, DN]])
        if scalar is None:
            nc.vector.tensor_tensor(out=out, in0=in0, in1=m_b, op=mul)
        else:
            nc.vector.scalar_tensor_tensor(
                out=out, in0=in0, scalar=scalar, in1=m_b, op0=mul, op1=mul)


def build_nc(n_iter):
    nc = bass.Bass()
    _install_compat(nc)
    mx = mybir.AluOpType.max
    mul = mybir.AluOpType.mult
    add = mybir.AluOpType.add

    goals_d = nc.declare_dram_parameter("goals", [128, 6, SLOT], DTYPE, isOutput=False)
    m_d = nc.declare_dram_parameter("m", [128, SLOT], DTYPE, isOutput=False)
    q0_d = nc.declare_dram_parameter("q0", [128, 6, SLOT], DTYPE, isOutput=True)
    q1_d = nc.declare_dram_parameter("q1", [128, 6, SLOT], DTYPE, isOutput=True)
    w_d = nc.declare_dram_parameter("w", [128, 6, SLOT], DTYPE, isOutput=True)

    with TileContext(nc) as tc:
        with tc.tile_pool(name="p", bufs=1) as pool:
            m_sb = pool.tile([128, SLOT], DTYPE)
            mg_sb = pool.tile([128, SLOT], DTYPE)     # gamma * m
            gme = pool.tile([128, 6, SLOT], DTYPE)
            v = pool.tile([128, 6, SLOT], DTYPE)      # holds gamma*V (prescaled)
            t0 = pool.tile([128, 6, SLOT], DTYPE)
            t1 = pool.tile([128, 6, SLOT], DTYPE)
            we = pool.tile([128, 8, SLOT], DTYPE)     # W slots 1..6, dups 0/7

            nc.gpsimd.memset(v[:], 0.0)
            nc.gpsimd.memset(we[:], 0.0)
            nc.sync.dma_start(out=m_sb[:], in_=m_d[:])
            goals_sb = pool.tile([128, 6, SLOT], DTYPE)
            # chunked goals DMA so the gm build overlaps the transfer
            nc.sync.dma_start(out=goals_sb[:, 0:3], in_=goals_d[:, 0:3])
            nc.sync.dma_start(out=goals_sb[:, 3:6], in_=goals_d[:, 3:6])

            # gm = goals * m (full slots incl halo rows; host fills halos)
            m_b3 = _rap(m_sb, 0, [[0, 3], [1, SLOT]])
            nc.vector.tensor_tensor(out=gme[:, 0:3], in0=goals_sb[:, 0:3],
                                    in1=m_b3, op=mul)
            nc.vector.tensor_tensor(out=gme[:, 3:6], in0=goals_sb[:, 3:6],
                                    in1=m_b3, op=mul)

            dv = (slice(None), slice(None), slice(DOF, DOF + DN))  # data view

            def halo_we():
                # half1 top halo <- half0 grid row 18 (buffer row 19)
                nc.sync.dma_start(out=we[64:128, 1:7, 0:26],
                                  in_=we[0:64, 1:7, 19 * 26:20 * 26])
                # half0 bottom halo <- half1 grid row 19 (buffer row 1)
                nc.sync.dma_start(out=we[0:64, 1:7, 20 * 26:21 * 26],
                                  in_=we[64:128, 1:7, 26:52])

            def w_build():
                # v holds gamma*V already, so W = v + gm. Boundary rows
                # (1, 19) first so the halo DMAs overlap the interior add.
                # rows 1 and 19 of every slot: [[546,6],[468,2],[1,26]]
                wa = _rap(we, SLOT + 26, [[SLOT, 6], [468, 2], [1, 26]])
                ga = _rap(v, 26, [[SLOT, 6], [468, 2], [1, 26]])
                ma = _rap(gme, 26, [[SLOT, 6], [468, 2], [1, 26]])
                nc.vector.tensor_tensor(out=wa, in0=ga, in1=ma, op=add)
                halo_we()
                # interior rows 2..18 (contiguous 442) of every slot
                wb = _rap(we, SLOT + 52, [[SLOT, 6], [1, 442]])
                gb = _rap(v, 52, [[SLOT, 6], [1, 442]])
                mb = _rap(gme, 52, [[SLOT, 6], [1, 442]])
                nc.vector.tensor_tensor(out=wb, in0=gb, in1=mb, op=add)

            def m2_build():
                # M2[o] = max(W[(o+1)%6], W[(o+5)%6]) -> t1, wrap-free in 3
                # ops (no dup-slot copies needed): o=1..4 batched, o=0, o=5.
                nc.vector.tensor_tensor(
                    out=t1[:, 1:5, DOF:DOF + DN], in0=we[:, 3:7, DOF:DOF + DN],
                    in1=we[:, 1:5, DOF:DOF + DN], op=mx)
                nc.vector.tensor_tensor(
                    out=t1[:, 0, DOF:DOF + DN], in0=we[:, 2, DOF:DOF + DN],
                    in1=we[:, 6, DOF:DOF + DN], op=mx)
                nc.vector.tensor_tensor(
                    out=t1[:, 5, DOF:DOF + DN], in0=we[:, 1, DOF:DOF + DN],
                    in1=we[:, 5, DOF:DOF + DN], op=mx)

            def t_build():
                _emit_shift_group(nc, t0, we, 1, D0, m_sb, None)
                _emit_shift_group(nc, t1, we, 1, D1, m_sb, None)

            def x_build():
                # X[o] = max(sh_{+d0(o)}(W[o]), sh_{-d0(o)}(W[o])) -> t0
                # (D1 = -D0, so max(T0,T1) = m * X)
                for oa, ob in PAIRS:
                    da, db = _delta(D0[oa]), _delta(D0[ob])
                    in0 = _rap(we, (oa + 1) * SLOT + DOF + da,
                               [[(ob - oa) * SLOT + (db - da), 2], [1, DN]])
                    in1 = _rap(we, (oa + 1) * SLOT + DOF - da,
                               [[(ob - oa) * SLOT - (db - da), 2], [1, DN]])
                    out = _rap(t0, oa * SLOT + DOF, [[(ob - oa) * SLOT, 2], [1, DN]])
                    nc.vector.tensor_tensor(out=out, in0=in0, in1=in1, op=mx)

            # mg = gamma*m once; the loop's final mask-mult then yields
            # gamma*V directly, eliminating a per-iter tensor_scalar.
            nc.vector.tensor_scalar_mul(out=mg_sb[:], in0=m_sb[:], scalar1=GAMMA)
            mg_b6d = _rap(mg_sb, DOF, [[0, 6], [1, DN]])

            for it in range(n_iter):
                w_build()
                x_build()
                m2_build()
                # Z = max(X, M2); v = (gamma*m) * Z
                nc.vector.tensor_tensor(out=t0[dv], in0=t0[dv], in1=t1[dv], op=mx)
                nc.vector.tensor_tensor(out=v[dv], in0=t0[dv], in1=mg_b6d, op=mul)

            # final partial: Q0 = m*sh0(W), Q1 = m*sh1(W), q2/q3 = W views
            w_build()
            nc.sync.dma_start(out=w_d[:], in_=we[:, 1:7])   # overlaps t_build
            t_build()
            nc.sync.dma_start(out=q0_d[:], in_=t0[:])
            nc.sync.dma_start(out=q1_d[:], in_=t1[:])
    return nc


_NC_CACHE = {}


def _get_nc(n_iter):
    if n_iter not in _NC_CACHE:
        _NC_CACHE[n_iter] = build_nc(n_iter)
    return _NC_CACHE[n_iter]


# ---------------------------------------------------------------- host side
def _to_padded_axial(x):
    out = np.zeros(x.shape[:-2] + (PR, PC), np.float32)
    out[..., _ROW, _COL] = x
    return out


def kernel(offset_input_goals, offset_current_state, offset_obstacles,
           num_iterations):
    global LAST_RESULT
    goals = np.asarray(offset_input_goals, np.float32)
    state = np.asarray(offset_current_state)
    obst = np.asarray(offset_obstacles, np.float32)
    n_iter = int(num_iterations)
    B = goals.shape[0]
    assert B == N_CORES * BPC and n_iter >= 1

    goals_ax = _to_padded_axial(goals)                     # [B,6,40,26]
    mask = _to_padded_axial(np.ones((E, E), np.float32))
    m_full = (1.0 - _to_padded_axial(obst)) * mask         # [B,40,26]

    def split(x):  # [B, ..., 40, 26] -> [B, 2, ..., 546]
        h0 = x[..., 0:21, :].reshape(x.shape[:-2] + (SLOT,))
        h1 = x[..., 19:40, :].reshape(x.shape[:-2] + (SLOT,))
        return h0, h1

    g0, g1 = split(goals_ax)
    m0, m1 = split(m_full)

    in_maps = []
    for c in range(N_CORES):
        s = slice(c * BPC, (c + 1) * BPC)
        in_maps.append({
            "goals": np.concatenate([g0[s], g1[s]], 0).astype(NP_DT),
            "m": np.concatenate([m0[s], m1[s]], 0).astype(NP_DT),
        })

    nc = _get_nc(n_iter)
    res = run_bass_kernel_spmd(nc, in_maps, core_ids=list(range(N_CORES)),
                               trace=TRACE)
    LAST_RESULT = res

    out = np.zeros((B, 4), np.float32)
    alpha = state[:, 0].astype(np.int64)
    uu = (state[:, 1] - state[:, 2] // 2 + ADD).astype(np.int64)
    vv = state[:, 2].astype(np.int64)
    rot = (alpha + 1) % 6
    h = (uu > 18).astype(np.int64)
    r = np.where(h == 0, uu + 1, uu - 18)
    idx = r * 26 + vv
    for c in range(N_CORES):
        rr = res.results[c]
        q0 = np.asarray(rr["q0"], np.float32)
        q1 = np.asarray(rr["q1"], np.float32)
        w = np.asarray(rr["w"], np.float32)
        bs = np.arange(c * BPC, (c + 1) * BPC)
        p = h[bs] * 64 + np.arange(BPC)
        out[bs, 0] = q0[p, rot[bs], idx[bs]]
        out[bs, 1] = q1[p, rot[bs], idx[bs]]
        out[bs, 2] = w[p, (rot[bs] + 1) % 6, idx[bs]]
        out[bs, 3] = w[p, (rot[bs] + 5) % 6, idx[bs]]
    return out


# revision 20
# speedup vs baseline: 1.4083x; 1.0185x over previous
"""Trainium2 Bass kernel for the CerealBar VIN problem.

Self-contained: hardcodes shapes B=512, E=25, 6 orientations, gamma=0.9,
8-core batch sharding (64 samples/core).

Math (derived from the reference, validated bit-exact in numpy):
  The grouped 3x3 conv is a set of one-hot spatial shifts. With
  m = (1-obstacles_axial) * axial_mask (0/1), gm[o] = goals_axial[o]*m:
    RT0[o] = m * sh_{d0(o)}(gm[o]);  RT1[o] = m * sh_{d1(o)}(gm[o])
    v0[o]  = max(RT0[o], RT1[o], gm[(o+1)%6], gm[(o+5)%6])
    repeat n-1 times:
      T0[o] = m * g * sh_{d0(o)}(v[o]);  T1[o] = m * g * sh_{d1(o)}(v[o])
      W[o]  = g*v[o] + gm[o]
      v[o]  = max(T0[o]+RT0[o], T1[o]+RT1[o], W[(o+1)%6], W[(o+5)%6])
    Q0 = T0+RT0, Q1 = T1+RT1, W = g*v+gm   (from final v)
    out[b] = [Q0[rot,uu,vv], Q1[rot,uu,vv], W[(rot+1)%6][uu,vv],
              W[(rot+5)%6][uu,vv]],  rot=(alpha+1)%6, uu=u-v//2+12

Device layout: partition p = h*64 + i -> sample i of the core, row-half h.
Each (orientation, half) plane = 21 rows x 26 cols flat (546): row 0 top
halo, rows 1..19 data, row 20 bottom halo, col 25 zero pad. half0 data =
grid rows 0..18, half1 = grid rows 19..37 (37 is a fake all-masked row).
Shifts are free-dim AP offsets (d = 26*dy+dx); the row-split halo rows are
refreshed once per iteration by two cross-partition SBUF-SBUF DMAs.
"""
import json
import sys

sys.path.insert(0, "/opt/trn_rl_repo")

import numpy as np

import concourse.bass as bass
import concourse.mybir as mybir
from concourse.ap import AP
from concourse.bass_utils import run_bass_kernel_spmd
from concourse.tile import TileContext

E = 25
ADD = 12
GAMMA = 0.9
PR = 40          # padded full-grid rows (grid rows -1..38 at idx r+1)
PC = 26
SLOT = 546       # 21 * 26 per half-plane
DOF = 26         # data offset within a slot (row 1)
DN = 494         # data elems (rows 1..19)
N_CORES = 8
BPC = 64         # samples per core

# shifts (dy, dx): out[y, x] = in[y+dy, x+dx]
D0 = [(0, 1), (1, 0), (1, -1), (0, -1), (-1, 0), (-1, 1)]
D1 = [(0, -1), (-1, 0), (-1, 1), (0, 1), (1, 0), (1, -1)]
PAIRS = [(0, 3), (1, 2), (4, 5)]  # (0,3) first: dy=0, no halo-row dep

import os as _os

if _os.environ.get("KDT", "fp16") == "fp16":
    DTYPE = mybir.dt.float16
    NP_DT = np.float16
else:
    DTYPE = mybir.dt.float32
    NP_DT = np.float32

TRACE = False
LAST_RESULT = None

_u = np.arange(E)[:, None]
_v = np.arange(E)[None, :]
_ROW = (_u - _v // 2 + ADD) + 1
_COL = np.broadcast_to(_v, (E, E))


# ---------------------------------------------------------------- BIR fixups
def _split_multi_waits(bir):
    """The installed walrus rejects >1 sync wait per instruction; hoist
    extras onto single-wait NoOps inserted before it on the same engine."""
    for fn in bir.get("functions", []):
        for blk in fn.get("blocks", []):
            out = []
            for ins in blk.get("instructions", []):
                si = ins.get("sync_info")
                waits = (si or {}).get("on_wait") or []
                if len(waits) > 1:
                    for k, w in enumerate(waits[:-1]):
                        out.append({
                            "debug": ins.get("debug", 0),
                            "engine": ins["engine"],
                            "ins": [], "outs": [],
                            "name": f"{ins['name']}_w{k}",
                            "opcode": "NoOp",
                            "sync_info": {"on_wait": [w], "on_update": []},
                            "text_hint": "split_wait",
                        })
                    si["on_wait"] = [waits[-1]]
                out.append(ins)
            blk["instructions"] = out
    return bir


def _install_compat(nc):
    orig = nc.to_json_bytes

    def patched():
        return json.dumps(_split_multi_waits(json.loads(orig()))).encode()

    nc.to_json_bytes = patched


# ---------------------------------------------------------------- kernel build
def _rap(t, off, pairs):
    """Raw AP over pool tile t (full 128 partitions) with free dims pairs."""
    return AP(t.tensor, int(t.offset) + off, [list(t.ap[0])] + [list(p) for p in pairs])


def _delta(d):
    return 26 * d[0] + d[1]


def _emit_shift_group(nc, dst, src, src_slot0, dlist, m_sb, scalar, rt=None):
    """dst[o] = (sh_{dlist[o]}(src[slot o+src_slot0]) * scalar) * m, for the
    three plane-pairs. If scalar is None, plain mult by m (RT build)."""
    mul = mybir.AluOpType.mult
    for oa, ob in PAIRS:
        da, db = _delta(dlist[oa]), _delta(dlist[ob])
        step = (ob - oa) * SLOT + (db - da)
        in0 = _rap(src, (oa + src_slot0) * SLOT + DOF + da, [[step, 2], [1, DN]])
        out = _rap(dst, oa * SLOT + DOF, [[(ob - oa) * SLOT, 2], [1, DN]])
        m_b = _rap(m_sb, DOF, [[0, 2], [1, DN]])
        if scalar is None:
            nc.vector.tensor_tensor(out=out, in0=in0, in1=m_b, op=mul)
        else:
            nc.vector.scalar_tensor_tensor(
                out=out, in0=in0, scalar=scalar, in1=m_b, op0=mul, op1=mul)


def build_nc(n_iter):
    nc = bass.Bass()
    _install_compat(nc)
    mx = mybir.AluOpType.max
    mul = mybir.AluOpType.mult
    add = mybir.AluOpType.add

    goals_d = nc.declare_dram_parameter("goals", [128, 6, SLOT], DTYPE, isOutput=False)
    m_d = nc.declare_dram_parameter("m", [128, SLOT], DTYPE, isOutput=False)
    q0_d = nc.declare_dram_parameter("q0", [128, 6, SLOT], DTYPE, isOutput=True)
    q1_d = nc.declare_dram_parameter("q1", [128, 6, SLOT], DTYPE, isOutput=True)
    w_d = nc.declare_dram_parameter("w", [128, 6, SLOT], DTYPE, isOutput=True)

    with TileContext(nc) as tc:
        with tc.tile_pool(name="p", bufs=1) as pool:
            m_sb = pool.tile([128, SLOT], DTYPE)
            mg_sb = pool.tile([128, SLOT], DTYPE)     # gamma * m
            gme = pool.tile([128, 6, SLOT], DTYPE)
            v = pool.tile([128, 6, SLOT], DTYPE)      # holds gamma*V (prescaled)
            t0 = pool.tile([128, 6, SLOT], DTYPE)
            t1 = pool.tile([128, 6, SLOT], DTYPE)
            we = pool.tile([128, 8, SLOT], DTYPE)     # W slots 1..6, dups 0/7

            nc.gpsimd.memset(v[:], 0.0)
            nc.gpsimd.memset(we[:], 0.0)
            nc.sync.dma_start(out=m_sb[:], in_=m_d[:])
            goals_sb = pool.tile([128, 6, SLOT], DTYPE)
            # chunked goals DMA so the gm build overlaps the transfer
            nc.sync.dma_start(out=goals_sb[:, 0:3], in_=goals_d[:, 0:3])
            nc.sync.dma_start(out=goals_sb[:, 3:6], in_=goals_d[:, 3:6])

            # gm = goals * m (full slots incl halo rows; host fills halos)
            m_b3 = _rap(m_sb, 0, [[0, 3], [1, SLOT]])
            nc.vector.tensor_tensor(out=gme[:, 0:3], in0=goals_sb[:, 0:3],
                                    in1=m_b3, op=mul)
            nc.vector.tensor_tensor(out=gme[:, 3:6], in0=goals_sb[:, 3:6],
                                    in1=m_b3, op=mul)

            dv = (slice(None), slice(None), slice(DOF, DOF + DN))  # data view

            def halo_we():
                # half1 top halo <- half0 grid row 18 (buffer row 19)
                nc.sync.dma_start(out=we[64:128, 1:7, 0:26],
                                  in_=we[0:64, 1:7, 19 * 26:20 * 26])
                # half0 bottom halo <- half1 grid row 19 (buffer row 1)
                nc.sync.dma_start(out=we[0:64, 1:7, 20 * 26:21 * 26],
                                  in_=we[64:128, 1:7, 26:52])

            def w_build():
                # v holds gamma*V already, so W = v + gm. Boundary rows
                # (1, 19) first so the halo DMAs overlap the interior add.
                # rows 1 and 19 of every slot: [[546,6],[468,2],[1,26]]
                wa = _rap(we, SLOT + 26, [[SLOT, 6], [468, 2], [1, 26]])
                ga = _rap(v, 26, [[SLOT, 6], [468, 2], [1, 26]])
                ma = _rap(gme, 26, [[SLOT, 6], [468, 2], [1, 26]])
                nc.vector.tensor_tensor(out=wa, in0=ga, in1=ma, op=add)
                halo_we()
                # interior rows 2..18 (contiguous 442) of every slot
                wb = _rap(we, SLOT + 52, [[SLOT, 6], [1, 442]])
                gb = _rap(v, 52, [[SLOT, 6], [1, 442]])
                mb = _rap(gme, 52, [[SLOT, 6], [1, 442]])
                nc.vector.tensor_tensor(out=wb, in0=gb, in1=mb, op=add)

            def m2_build():
                # M2[o] = max(W[(o+1)%6], W[(o+5)%6]) -> t1, wrap-free in 3
                # ops (no dup-slot copies needed): o=1..4 batched, o=0, o=5.
                nc.vector.tensor_tensor(
                    out=t1[:, 1:5, DOF:DOF + DN], in0=we[:, 3:7, DOF:DOF + DN],
                    in1=we[:, 1:5, DOF:DOF + DN], op=mx)
                # o=0 and o=5 wrap cases in one negative-stride op:
                # t1[{0,5}] = max(we[{2,1}], we[{6,5}])
                nc.vector.tensor_tensor(
                    out=_rap(t1, DOF, [[5 * SLOT, 2], [1, DN]]),
                    in0=_rap(we, 2 * SLOT + DOF, [[-SLOT, 2], [1, DN]]),
                    in1=_rap(we, 6 * SLOT + DOF, [[-SLOT, 2], [1, DN]]), op=mx)

            def t_build():
                _emit_shift_group(nc, t0, we, 1, D0, m_sb, None)
                _emit_shift_group(nc, t1, we, 1, D1, m_sb, None)

            def x_build():
                # X[o] = max(sh_{+d0(o)}(W[o]), sh_{-d0(o)}(W[o])) -> t0
                # (D1 = -D0, so max(T0,T1) = m * X)
                for oa, ob in PAIRS:
                    da, db = _delta(D0[oa]), _delta(D0[ob])
                    in0 = _rap(we, (oa + 1) * SLOT + DOF + da,
                               [[(ob - oa) * SLOT + (db - da), 2], [1, DN]])
                    in1 = _rap(we, (oa + 1) * SLOT + DOF - da,
                               [[(ob - oa) * SLOT - (db - da), 2], [1, DN]])
                    out = _rap(t0, oa * SLOT + DOF, [[(ob - oa) * SLOT, 2], [1, DN]])
                    nc.vector.tensor_tensor(out=out, in0=in0, in1=in1, op=mx)

            # mg = gamma*m once; the loop's final mask-mult then yields
            # gamma*V directly, eliminating a per-iter tensor_scalar.
            nc.vector.tensor_scalar_mul(out=mg_sb[:], in0=m_sb[:], scalar1=GAMMA)
            mg_b6d = _rap(mg_sb, DOF, [[0, 6], [1, DN]])

            for it in range(n_iter):
                w_build()
                x_build()
                m2_build()
                # Z = max(X, M2); v = (gamma*m) * Z
                nc.vector.tensor_tensor(out=t0[dv], in0=t0[dv], in1=t1[dv], op=mx)
                nc.vector.tensor_tensor(out=v[dv], in0=t0[dv], in1=mg_b6d, op=mul)

            # final partial: Q0 = m*sh0(W), Q1 = m*sh1(W), q2/q3 = W views
            w_build()
            nc.sync.dma_start(out=w_d[:], in_=we[:, 1:7])   # overlaps t_build
            t_build()
            nc.sync.dma_start(out=q0_d[:], in_=t0[:])
            nc.sync.dma_start(out=q1_d[:], in_=t1[:])
    return nc


_NC_CACHE = {}


def _get_nc(n_iter):
    if n_iter not in _NC_CACHE:
        _NC_CACHE[n_iter] = build_nc(n_iter)
    return _NC_CACHE[n_iter]


# ---------------------------------------------------------------- host side
def _to_padded_axial(x):
    out = np.zeros(x.shape[:-2] + (PR, PC), np.float32)
    out[..., _ROW, _COL] = x
    return out


def kernel(offset_input_goals, offset_current_state, offset_obstacles,
           num_iterations):
    global LAST_RESULT
    goals = np.asarray(offset_input_goals, np.float32)
    state = np.asarray(offset_current_state)
    obst = np.asarray(offset_obstacles, np.float32)
    n_iter = int(num_iterations)
    B = goals.shape[0]
    assert B == N_CORES * BPC and n_iter >= 1

    goals_ax = _to_padded_axial(goals)                     # [B,6,40,26]
    mask = _to_padded_axial(np.ones((E, E), np.float32))
    m_full = (1.0 - _to_padded_axial(obst)) * mask         # [B,40,26]

    def split(x):  # [B, ..., 40, 26] -> [B, 2, ..., 546]
        h0 = x[..., 0:21, :].reshape(x.shape[:-2] + (SLOT,))
        h1 = x[..., 19:40, :].reshape(x.shape[:-2] + (SLOT,))
        return h0, h1

    g0, g1 = split(goals_ax)
    m0, m1 = split(m_full)

    in_maps = []
    for c in range(N_CORES):
        s = slice(c * BPC, (c + 1) * BPC)
        in_maps.append({
            "goals": np.concatenate([g0[s], g1[s]], 0).astype(NP_DT),
            "m": np.concatenate([m0[s], m1[s]], 0).astype(NP_DT),
        })

    nc = _get_nc(n_iter)
    res = run_bass_kernel_spmd(nc, in_maps, core_ids=list(range(N_CORES)),
                               trace=TRACE)
    LAST_RESULT = res

    out = np.zeros((B, 4), np.float32)
    alpha = state[:, 0].astype(np.int64)
    uu = (state[:, 1] - state[:, 2] // 2 + ADD).astype(np.int64)
    vv = state[:, 2].astype(np.int64)
    rot = (alpha + 1) % 6
    h = (uu > 18).astype(np.int64)
    r = np.where(h == 0, uu + 1, uu - 18)
    idx = r * 26 + vv
    for c in range(N_CORES):
        rr = res.results[c]
        q0 = np.asarray(rr["q0"], np.float32)
        q1 = np.asarray(rr["q1"], np.float32)
        w = np.asarray(rr["w"], np.float32)
        bs = np.arange(c * BPC, (c + 1) * BPC)
        p = h[bs] * 64 + np.arange(BPC)
        out[bs, 0] = q0[p, rot[bs], idx[bs]]
        out[bs, 1] = q1[p, rot[bs], idx[bs]]
        out[bs, 2] = w[p, (rot[bs] + 1) % 6, idx[bs]]
        out[bs, 3] = w[p, (rot[bs] + 5) % 6, idx[bs]]
    return out
